# revision 2
# baseline (speedup 1.0000x reference)
"""Trainium2 Bass kernel v2.1 for nn_FFFFanout (moe_routing tree-MLP).

Contract: kernel(**inputs) takes FULL unsharded numpy inputs
  oldx  [2, 2048, 1024] f32,  W_in [21840, 1024] f32,
  b_in  [21840] f32,          W_out [1024, 21840] f32
returns [2, 2048, 1024] f32.

v2 architecture — exploit the tree sparsity instead of dense matmuls:
  * Only tree levels 0-4 (341 of 1365 groups per p, 25% of HID) are ever
    read by the routing argmax; level-5 groups (75% of HID) contribute to
    the output only for the ONE selected group per (token, p).
  * Phase A (PE): dense matmul1 + gelu over just the 44 route tiles
    (f-major planes, rows packed p*341+g). Routing needs ~2^-15 logit
    precision (measured flip tolerance): 3-pass bf16 hi/lo. (fp32r
    1-pass measured 2^-13.5 -> 9 flipped paths -> 2.3e-2 rel err: fails.)
  * Cascade (token-major): PE-transpose dec tiles to [token, group], then
    per (p, chunk) walk the tree with small DVE ops:
    onehot(cur) = is_equal(iota, cur); m = accum(onehot * (dec+1));
    cur' = 4*cur + m. Yields the L0-4 mask + the selected level-5 group
    index as a value, already token-major for the gather index table.
  * Gather (gpsimd dma_gather): fetch only the selected level-5 W_in rows
    (4x1024 + bias) and W_out columns by int16 index; fused DVE
    multiply-reduce (accum_out) / multiply-accumulate, token-major.
  * Phase C (PE): dense matmul2 over the 44 route tiles with masked acts
    stationary, accumulating out[token, D] in PSUM; add the gather
    contribution; DMA out as [T, D] (no transposes anywhere).

v2.1 scheduling: per-p cascades + gather issues interleave INTO phase A
(dec transposes lag one subtile), gather DVE consumers are placed in the
DVE stream where their data has landed, and phase C overlaps the gather
tail. DVE uses tensor_tensor (2x) + tensor_scalar (4x, accum_out) forms
instead of scalar_tensor_tensor (which has no fast mode).
"""
import sys

if "/opt/trn_rl_repo" not in sys.path:
    sys.path.insert(0, "/opt/trn_rl_repo")

from contextlib import ExitStack

import ml_dtypes
import numpy as np

import concourse.bass as bass  # noqa: F401
import concourse.mybir as mybir
import concourse.tile as tile
from concourse import bacc
from concourse.bass_utils import run_bass_kernel_spmd
from concourse.library_config import mlp as MLP_LIB
from concourse.library_config import standard as STD_LIB

F32 = mybir.dt.float32
F32R = mybir.dt.float32r
BF16 = mybir.dt.bfloat16
I16 = mybir.dt.int16
U8 = mybir.dt.uint8
NPBF16 = ml_dtypes.bfloat16
ALU = mybir.AluOpType
ACT = mybir.ActivationFunctionType

D = 1024
P = 4
DEPTH = 5
FAN = 4
G = 1365
RG = 341                  # route groups per p (levels 0-4)
RROWS = P * RG            # 1364 packed rows per f-plane
NS = 11                   # 128-row subtiles per f-plane
PLANE = NS * 128          # 1408
NRT = FAN * NS            # 44 route tiles
L5 = 1024                 # level-5 groups per p
E5 = 4224                 # gather elem: 4*1024 W row + 4 bias + 124 pad
T = 512
KC = D // 128             # 8
NCORES = 8
PLAT = [0, 1, 5, 21, 85, 341]

ROUTE_MODE = "hilo3"      # "hilo3" (3-pass bf16) or "f32r" (1-pass fp32r)
DEBUG = False


def build_nc(route_mode=ROUTE_MODE, debug=DEBUG):
    nc = bacc.Bacc("TRN2", target_bir_lowering=False, debug=False,
                   num_devices=NCORES)

    if route_mode == "f32r":
        xTd = nc.dram_tensor("xT", [D, T], F32R, kind="ExternalInput").ap()
        Wrd = nc.dram_tensor("Wr", [D, NRT * 128], F32R,
                             kind="ExternalInput").ap()
    else:
        xhid = nc.dram_tensor("xhi", [D, T], BF16, kind="ExternalInput").ap()
        xlod = nc.dram_tensor("xlo", [D, T], BF16, kind="ExternalInput").ap()
        Whid = nc.dram_tensor("Whi", [D, NRT * 128], BF16,
                              kind="ExternalInput").ap()
        Wlod = nc.dram_tensor("Wlo", [D, NRT * 128], BF16,
                              kind="ExternalInput").ap()
    xtokd = nc.dram_tensor("xtok", [T, D], BF16, kind="ExternalInput").ap()
    bvecd = nc.dram_tensor("bvec", [128, NRT], F32, kind="ExternalInput").ap()
    WoTd = nc.dram_tensor("WoT", [NRT * 128, D], BF16,
                          kind="ExternalInput").ap()
    W5ind = nc.dram_tensor("W5in", [P * L5, E5], BF16,
                           kind="ExternalInput").ap()
    W5od = nc.dram_tensor("W5o", [P * L5, E5], BF16,
                          kind="ExternalInput").ap()
    identd = nc.dram_tensor("ident", [128, 128], BF16,
                            kind="ExternalInput").ap()
    iotad = nc.dram_tensor("iota", [128, RG], F32, kind="ExternalInput").ap()
    outd = nc.dram_tensor("outT", [T, D], F32, kind="ExternalOutput").ap()
    if debug:
        didx = nc.dram_tensor("didx", [128, 16], F32,
                              kind="ExternalOutput").ap()
        dmask = nc.dram_tensor("dmask", [128, NS, T], BF16,
                               kind="ExternalOutput").ap()

    with tile.TileContext(nc) as tc, ExitStack() as top:
        perm = top.enter_context(tc.tile_pool(name="perm", bufs=1))
        dramp = top.enter_context(tc.tile_pool(name="dram", bufs=1,
                                               space="DRAM"))

        nc.gpsimd.load_library(MLP_LIB)

        acr = perm.tile([128, NRT, T], BF16)       # route acts (output path)
        bt = perm.tile([128, NRT], F32)
        idt = perm.tile([128, 128], BF16)
        iot = perm.tile([128, RG], F32)
        xtok = perm.tile([128, 4, D], BF16)
        mask_g = perm.tile([128, NS, T], BF16)     # group-major L0-4 mask
        prodt = perm.tile([128, D], BF16)          # dot/accum staging
        idxf = perm.tile([128, 16], F32)
        idxi = perm.tile([128, 16], I16)
        idxt = perm.tile([128, 128], I16)
        red5 = perm.tile([128, 64], F32)           # (p,c,f) dot accums
        act5 = perm.tile([128, 64], F32)           # gelu'd level-5 acts
        out5 = perm.tile([128, 4, D], BF16)        # gather contribution
        dscr = dramp.tile([16, 128], I16, name="dscr")

        nc.sync.dma_start(bt[:], bvecd[:])
        nc.sync.dma_start(idt[:], identd[:])
        nc.sync.dma_start(iot[:], iotad[:])
        nc.sync.dma_start(
            xtok[:], xtokd.rearrange("(c p) d -> p c d", p=128))
        # junk rows 84..127 of the last mask_g subtile never get written by
        # the (84-col) transpose evac; zero them once so 0*garbage != NaN
        nc.gpsimd.memset(mask_g[:], 0.0)

        with ExitStack() as gstk:
            g5p = gstk.enter_context(tc.tile_pool(name="g5", bufs=4))
            gtile = {}

            # ============ phase A + cascades + gather issue ============
            pa = gstk.enter_context(ExitStack())
            wrp = pa.enter_context(tc.tile_pool(name="wr", bufs=2))
            a32p = pa.enter_context(tc.tile_pool(name="a32", bufs=4))
            tmpp = pa.enter_context(tc.tile_pool(name="tmp", bufs=1))
            decp = pa.enter_context(tc.tile_pool(name="decs", bufs=2))
            cascp = pa.enter_context(tc.tile_pool(name="casc", bufs=1))
            psA = pa.enter_context(tc.tile_pool(name="psA", bufs=4,
                                                space="PSUM"))
            psT = pa.enter_context(tc.tile_pool(name="psT", bufs=4,
                                                space="PSUM"))

            dec1t = cascp.tile([128, 4, RROWS], BF16)  # token-major dec + 1
            mask_tok = cascp.tile([128, 4, RROWS], BF16)
            curt = cascp.tile([128, P * 4 * (DEPTH + 1)], F32)
            macc = cascp.tile([128, P * 4 * DEPTH], F32)
            nc.gpsimd.memset(mask_tok[:], 0.0)

            if route_mode == "f32r":
                xsb = cascp.tile([128, KC, T], F32R)
                for c in range(KC):
                    nc.sync.dma_start(xsb[:, c, :],
                                      xTd[c * 128:(c + 1) * 128, :])
            else:
                xh = cascp.tile([128, KC, T], BF16)
                xl = cascp.tile([128, KC, T], BF16)
                for c in range(KC):
                    nc.sync.dma_start(xh[:, c, :],
                                      xhid[c * 128:(c + 1) * 128, :])
                    nc.sync.dma_start(xl[:, c, :],
                                      xlod[c * 128:(c + 1) * 128, :])

            dec_tiles = {}
            quad = {}

            def emit_dec(s):
                a0, a1, a2, a3 = (quad.pop(f) for f in range(4))
                m01 = tmpp.tile([128, T], F32, tag="m01")
                m23 = tmpp.tile([128, T], F32, tag="m23")
                nc.vector.tensor_tensor(m01[:], a0[:], a1[:], ALU.max)
                nc.vector.tensor_tensor(m23[:], a2[:], a3[:], ALU.max)
                b1t = tmpp.tile([128, T], U8, tag="b1")
                nc.vector.tensor_tensor(b1t[:], m23[:], m01[:], ALU.is_gt)
                b01 = tmpp.tile([128, T], BF16, tag="b01")
                nc.vector.tensor_tensor(b01[:], a1[:], a0[:], ALU.is_gt)
                b23 = tmpp.tile([128, T], F32, tag="m23")
                nc.vector.tensor_tensor(b23[:], a3[:], a2[:], ALU.is_gt)
                nc.vector.select(m01[:], b1t[:], b23[:], b01[:])
                ds = dec_tiles[s]
                nc.vector.scalar_tensor_tensor(
                    ds[:], b1t[:], 2.0, m01[:], op0=ALU.mult, op1=ALU.add)

            def emit_route(f, s):
                t_lin = f * NS + s
                if f == 0:
                    dec_tiles[s] = decp.tile([128, T], BF16, tag="dec",
                                             name=f"dec_{s}")
                if route_mode == "f32r":
                    wr = wrp.tile([128, KC, 128], F32R, tag="wr",
                                  name=f"wr_{t_lin}")
                    nc.sync.dma_start(
                        wr[:],
                        Wrd[:, t_lin * 128:(t_lin + 1) * 128]
                        .rearrange("(c p) h -> p c h", p=128))
                    ps = psA.tile([128, T], F32, tag="ps", name=f"ps_{t_lin}")
                    for c in range(KC):
                        nc.tensor.matmul(ps[:], wr[:, c, :], xsb[:, c, :],
                                         start=(c == 0), stop=(c == KC - 1))
                else:
                    wh = wrp.tile([128, KC, 128], BF16, tag="wh",
                                  name=f"wh_{t_lin}")
                    wl = wrp.tile([128, KC, 128], BF16, tag="wl",
                                  name=f"wl_{t_lin}")
                    nc.sync.dma_start(
                        wh[:],
                        Whid[:, t_lin * 128:(t_lin + 1) * 128]
                        .rearrange("(c p) h -> p c h", p=128))
                    nc.sync.dma_start(
                        wl[:],
                        Wlod[:, t_lin * 128:(t_lin + 1) * 128]
                        .rearrange("(c p) h -> p c h", p=128))
                    ps = psA.tile([128, T], F32, tag="ps", name=f"ps_{t_lin}")
                    for c in range(KC):
                        nc.tensor.matmul(ps[:], wh[:, c, :], xh[:, c, :],
                                         start=(c == 0), stop=False)
                    for c in range(KC):
                        nc.tensor.matmul(ps[:], wl[:, c, :], xh[:, c, :],
                                         start=False, stop=False)
                    for c in range(KC):
                        nc.tensor.matmul(ps[:], wh[:, c, :], xl[:, c, :],
                                         start=False, stop=(c == KC - 1))
                a = a32p.tile([128, T], F32, tag="a", name=f"a_{t_lin}")
                nc.scalar.activation(a[:], ps[:], ACT.Gelu,
                                     bias=bt[:, t_lin:t_lin + 1], scale=1.0)
                nc.scalar.activation(acr[:, t_lin, :], ps[:], ACT.Gelu,
                                     bias=bt[:, t_lin:t_lin + 1], scale=1.0)
                quad[f] = a
                if f == FAN - 1:
                    emit_dec(s)

            def emit_ttrans(s):
                # dec transpose -> token-major dec+1 (scalar Copy, bias=1)
                ncol = min(128, RROWS - s * 128)
                for c in range(4):
                    pt = psT.tile([128, 128], BF16, tag="pt",
                                  name=f"ptd_{s}_{c}")
                    nc.tensor.transpose(
                        pt[:], dec_tiles[s][:, c * 128:(c + 1) * 128], idt[:])
                    nc.scalar.activation(
                        dec1t[:, c, s * 128:s * 128 + ncol], pt[:, :ncol],
                        ACT.Copy, bias=1.0)

            def emit_casc(p):
                base = p * RG
                for c in range(4):
                    pc6 = (p * 4 + c) * (DEPTH + 1)
                    pc5 = (p * 4 + c) * DEPTH
                    nc.vector.memset(mask_tok[:, c, base:base + 1], 1.0)
                    nc.vector.tensor_copy(curt[:, pc6 + 1:pc6 + 2],
                                          dec1t[:, c, base:base + 1])
                    for d in range(1, DEPTH):
                        lo, hi = PLAT[d], PLAT[d + 1]
                        n = hi - lo
                        cur = curt[:, pc6 + d:pc6 + d + 1]
                        nxt = curt[:, pc6 + d + 1:pc6 + d + 2]
                        mslice = mask_tok[:, c, base + lo:base + hi]
                        nc.vector.tensor_scalar(
                            mslice, iot[:, lo:hi], cur, None, op0=ALU.is_equal)
                        nc.vector.scalar_tensor_tensor(
                            prodt[:, :n], mslice, 1.0,
                            dec1t[:, c, base + lo:base + hi],
                            op0=ALU.mult, op1=ALU.mult,
                            accum_out=macc[:, pc5 + d:pc5 + d + 1])
                        nc.vector.scalar_tensor_tensor(
                            nxt, cur, 4.0, macc[:, pc5 + d:pc5 + d + 1],
                            op0=ALU.mult, op1=ALU.add)
                    nc.vector.tensor_scalar(
                        idxf[:, p * 4 + c:p * 4 + c + 1],
                        curt[:, pc6 + DEPTH:pc6 + DEPTH + 1],
                        float(1024 * p - RG), None, op0=ALU.add)

            def emit_idx(p):
                nc.vector.tensor_copy(idxi[:, 4 * p:4 * p + 4],
                                      idxf[:, 4 * p:4 * p + 4])
                nc.sync.dma_start(
                    dscr[4 * p:4 * p + 4, :].rearrange("m t -> t m"),
                    idxi[:, 4 * p:4 * p + 4])
                for r in range(8):
                    nc.scalar.dma_start(
                        idxt[16 * r:16 * r + 16, 32 * p:32 * p + 32],
                        dscr[4 * p:4 * p + 4, :]
                        .rearrange("m (k j) -> j (m k)", k=8))

            def emit_gather(p):
                for kind, src in (("i", W5ind), ("o", W5od)):
                    for h in range(2):
                        g = g5p.tile([128, 2, E5], BF16, tag="g5",
                                     name=f"g{kind}_{p}_{h}")
                        nc.gpsimd.dma_gather(
                            g[:], src[:],
                            idxt[:, 32 * p + 16 * h:32 * p + 16 * h + 16],
                            256, 256, E5)
                        gtile[(kind, p, h)] = g

            def emit_dots(p, h):
                # act5[t, p*16+c*4+f] = gelu(sum_d W5in[sel] * x + bias)
                g = gtile[("i", p, h)]
                for hc in range(2):
                    c = 2 * h + hc
                    for f in range(FAN):
                        r0 = p * 16 + c * 4 + f
                        nc.vector.scalar_tensor_tensor(
                            prodt[:], g[:, hc, f * D:(f + 1) * D], 1.0,
                            xtok[:, c, :], op0=ALU.mult, op1=ALU.mult,
                            accum_out=red5[:, r0:r0 + 1])
                    rs = red5[:, p * 16 + c * 4:p * 16 + c * 4 + 4]
                    nc.vector.tensor_tensor(
                        rs, rs, g[:, hc, FAN * D:FAN * D + 4], ALU.add)
                    nc.scalar.activation(
                        act5[:, p * 16 + c * 4:p * 16 + c * 4 + 4], rs,
                        ACT.Gelu, bias=0.0, scale=1.0)

            def emit_accums(p, h):
                g = gtile[("o", p, h)]
                for hc in range(2):
                    c = 2 * h + hc
                    for f in range(FAN):
                        sc = act5[:, p * 16 + c * 4 + f:
                                  p * 16 + c * 4 + f + 1]
                        gsl = g[:, hc, f * D:(f + 1) * D]
                        if p == 0 and f == 0:
                            nc.vector.tensor_scalar(
                                out5[:, c, :], gsl, sc, None, op0=ALU.mult)
                        else:
                            nc.vector.scalar_tensor_tensor(
                                out5[:, c, :], gsl, sc, out5[:, c, :],
                                op0=ALU.mult, op1=ALU.add)

            # -------- the interleaved schedule --------
            def casc_issue(p):
                emit_casc(p)
                emit_idx(p)
                emit_gather(p)

            SCHED = {4: [lambda: casc_issue(0)],
                     5: [lambda: emit_dots(0, 0)],
                     6: [lambda: emit_dots(0, 1), lambda: emit_accums(0, 0)],
                     7: [lambda: emit_accums(0, 1), lambda: casc_issue(1)],
                     8: [lambda: emit_dots(1, 0), lambda: emit_dots(1, 1)],
                     9: [lambda: emit_accums(1, 0), lambda: emit_accums(1, 1),
                         lambda: casc_issue(2)],
                     10: [lambda: emit_dots(2, 0), lambda: emit_dots(2, 1)]}
            for s in range(NS):
                for f in range(FAN):
                    emit_route(f, s)
                if s >= 1:
                    emit_ttrans(s - 1)
                for fn in SCHED.get(s, []):
                    fn()
            emit_ttrans(NS - 1)
            emit_accums(2, 0)
            emit_accums(2, 1)
            casc_issue(3)
            if debug:
                nc.sync.dma_start(didx[:], idxf[:])

            # mask transposes -> group-major (psT reused)
            for s in range(NS):
                ncol = min(128, RROWS - s * 128)
                for c in range(4):
                    pt = psT.tile([128, 128], BF16, tag="pt",
                                  name=f"ptm_{s}_{c}")
                    nc.tensor.transpose(
                        pt[0:ncol, :],
                        mask_tok[:, c, s * 128:s * 128 + ncol], idt[:])
                    nc.scalar.activation(
                        mask_g[0:ncol, s, c * 128:(c + 1) * 128],
                        pt[0:ncol, :], ACT.Copy, bias=0.0)
            if debug:
                nc.sync.dma_start(dmask[:], mask_g[:])

            pa.close()

            # ============ phase C + gather tail + combine ============
            with ExitStack() as pc:
                wop = pc.enter_context(tc.tile_pool(name="wo", bufs=4))
                mkp = pc.enter_context(tc.tile_pool(name="mk", bufs=6))
                outp = pc.enter_context(tc.tile_pool(name="outp", bufs=1))
                psC = pc.enter_context(tc.tile_pool(name="psC", bufs=1,
                                                    space="PSUM"))
                cps = psC.tile([128, 4, D], F32)
                i = 0
                for f in range(FAN):
                    for s0 in range(0, NS, 2):
                        nt = min(2, NS - s0)
                        t0 = f * NS + s0
                        wo = wop.tile([128, nt, D], BF16, tag="wo",
                                      name=f"wo_{t0}")
                        nc.sync.dma_start(
                            wo[:],
                            WoTd[t0 * 128:(t0 + nt) * 128, :]
                            .rearrange("(u p) d -> p u d", p=128))
                        for u in range(nt):
                            s = s0 + u
                            mk = mkp.tile([128, T], BF16, tag="mk",
                                          name=f"mk_{t0 + u}")
                            nc.vector.tensor_tensor(
                                mk[:], acr[:, f * NS + s, :],
                                mask_g[:, s, :], ALU.mult)
                            for c in range(4):
                                for hh in range(2):
                                    nc.tensor.matmul(
                                        cps[:, c, hh * 512:(hh + 1) * 512],
                                        mk[:, c * 128:(c + 1) * 128],
                                        wo[:, u, hh * 512:(hh + 1) * 512],
                                        start=(i == 0), stop=(i == NRT - 1))
                            i += 1

                emit_dots(3, 0)
                emit_dots(3, 1)
                emit_accums(3, 0)
                emit_accums(3, 1)

                osb = outp.tile([128, 4, D], F32)
                for c in range(4):
                    nc.vector.tensor_tensor(osb[:, c, :], cps[:, c, :],
                                            out5[:, c, :], ALU.add)
                nc.sync.dma_start(outd.rearrange("(c p) d -> p c d", p=128),
                                  osb[:])

    nc.compile()
    return nc


_NC_CACHE = {}


def _get_nc(route_mode=ROUTE_MODE, debug=DEBUG):
    key = (route_mode, debug)
    if key not in _NC_CACHE:
        _NC_CACHE[key] = build_nc(route_mode, debug)
    return _NC_CACHE[key]


def _split_hi_lo(a):
    hi = a.astype(NPBF16)
    lo = (a - hi.astype(np.float32)).astype(NPBF16)
    return hi, lo


def _prep_inputs(oldx, W_in, b_in, W_out, route_mode):
    x = np.ascontiguousarray(np.asarray(oldx, np.float32).reshape(-1, D))
    Wr4 = np.asarray(W_in, np.float32).reshape(P, G, FAN, D)
    br = np.asarray(b_in, np.float32).reshape(P, G, FAN)
    Wo4 = np.asarray(W_out, np.float32).reshape(D, P, G, FAN)

    A = np.zeros((FAN, PLANE, D), np.float32)
    A[:, :RROWS] = Wr4[:, :RG].transpose(2, 0, 1, 3).reshape(FAN, RROWS, D)
    WrT = np.ascontiguousarray(A.reshape(FAN * PLANE, D).T)  # [D, 5632]

    b_dev = np.zeros((FAN, PLANE), np.float32)
    b_dev[:, :RROWS] = br[:, :RG].transpose(2, 0, 1).reshape(FAN, RROWS)
    bvec = np.ascontiguousarray(b_dev.reshape(NRT, 128).T)   # [128, 44]

    Wo_dev = np.zeros((FAN, PLANE, D), np.float32)
    Wo_dev[:, :RROWS] = (Wo4[:, :, :RG, :].transpose(3, 1, 2, 0)
                         .reshape(FAN, RROWS, D))
    WoT = np.ascontiguousarray(Wo_dev.reshape(FAN * PLANE, D)).astype(NPBF16)

    W5in = np.zeros((P * L5, E5), np.float32)
    W5in[:, :FAN * D] = Wr4[:, RG:].reshape(P * L5, FAN * D)
    W5in[:, FAN * D:FAN * D + FAN] = br[:, RG:].reshape(P * L5, FAN)
    W5in = W5in.astype(NPBF16)
    W5o = np.zeros((P * L5, E5), np.float32)
    W5o[:, :FAN * D] = (Wo4[:, :, RG:, :].transpose(1, 2, 3, 0)
                        .reshape(P * L5, FAN * D))
    W5o = W5o.astype(NPBF16)

    iota = np.tile(np.arange(RG, dtype=np.float32), (128, 1))
    ident = np.eye(128, dtype=np.float32).astype(NPBF16)
    xT = np.ascontiguousarray(x.T)                            # [D, B]

    shared = {"bvec": bvec, "WoT": WoT, "W5in": W5in, "W5o": W5o,
              "ident": ident, "iota": iota}
    in_maps = []
    for cc in range(NCORES):
        m = dict(shared)
        xs = np.ascontiguousarray(xT[:, cc * T:(cc + 1) * T])
        m["xtok"] = np.ascontiguousarray(x[cc * T:(cc + 1) * T, :]
                                         ).astype(NPBF16)
        if route_mode == "f32r":
            m["xT"] = xs
            m["Wr"] = WrT
        else:
            m["xhi"], m["xlo"] = _split_hi_lo(xs)
            m["Whi"], m["Wlo"] = _split_hi_lo(WrT)
        in_maps.append(m)
    return in_maps


_WARM = False


def run(oldx, W_in, b_in, W_out, trace=False, route_mode=ROUTE_MODE,
        debug=DEBUG):
    nc = _get_nc(route_mode, debug)
    in_maps = _prep_inputs(oldx, W_in, b_in, W_out, route_mode)
    global _WARM
    if not _WARM:
        run_bass_kernel_spmd(nc, in_maps, list(range(NCORES)), trace=False)
        _WARM = True
    res = run_bass_kernel_spmd(nc, in_maps, list(range(NCORES)), trace=trace)
    out = np.concatenate([res.results[c]["outT"] for c in range(NCORES)],
                         axis=0)
    return np.ascontiguousarray(out).reshape(np.asarray(oldx).shape), res


def kernel(oldx, W_in, b_in, W_out):
    out, _ = run(oldx, W_in, b_in, W_out, trace=False)
    return out


# revision 3
# speedup vs baseline: 1.0110x; 1.0110x over previous
"""Trainium2 Bass kernel v2.1 for nn_FFFFanout (moe_routing tree-MLP).

Contract: kernel(**inputs) takes FULL unsharded numpy inputs
  oldx  [2, 2048, 1024] f32,  W_in [21840, 1024] f32,
  b_in  [21840] f32,          W_out [1024, 21840] f32
returns [2, 2048, 1024] f32.

v2 architecture — exploit the tree sparsity instead of dense matmuls:
  * Only tree levels 0-4 (341 of 1365 groups per p, 25% of HID) are ever
    read by the routing argmax; level-5 groups (75% of HID) contribute to
    the output only for the ONE selected group per (token, p).
  * Phase A (PE): dense matmul1 + gelu over just the 44 route tiles
    (f-major planes, rows packed p*341+g). Routing needs ~2^-15 logit
    precision (measured flip tolerance): 3-pass bf16 hi/lo. (fp32r
    1-pass measured 2^-13.5 -> 9 flipped paths -> 2.3e-2 rel err: fails.)
  * Cascade (token-major): PE-transpose dec tiles to [token, group], then
    per (p, chunk) walk the tree with small DVE ops:
    onehot(cur) = is_equal(iota, cur); m = accum(onehot * (dec+1));
    cur' = 4*cur + m. Yields the L0-4 mask + the selected level-5 group
    index as a value, already token-major for the gather index table.
  * Gather (gpsimd dma_gather): fetch only the selected level-5 W_in rows
    (4x1024 + bias) and W_out columns by int16 index; fused DVE
    multiply-reduce (accum_out) / multiply-accumulate, token-major.
  * Phase C (PE): dense matmul2 over the 44 route tiles with masked acts
    stationary, accumulating out[token, D] in PSUM; add the gather
    contribution; DMA out as [T, D] (no transposes anywhere).

v2.1 scheduling: per-p cascades + gather issues interleave INTO phase A
(dec transposes lag one subtile), gather DVE consumers are placed in the
DVE stream where their data has landed, and phase C overlaps the gather
tail. DVE uses tensor_tensor (2x) + tensor_scalar (4x, accum_out) forms
instead of scalar_tensor_tensor (which has no fast mode).
"""
import sys

if "/opt/trn_rl_repo" not in sys.path:
    sys.path.insert(0, "/opt/trn_rl_repo")

from contextlib import ExitStack

import ml_dtypes
import numpy as np

import concourse.bass as bass  # noqa: F401
import concourse.mybir as mybir
import concourse.tile as tile
from concourse import bacc
from concourse.bass_utils import run_bass_kernel_spmd
from concourse.library_config import mlp as MLP_LIB
from concourse.library_config import standard as STD_LIB

F32 = mybir.dt.float32
F32R = mybir.dt.float32r
BF16 = mybir.dt.bfloat16
I16 = mybir.dt.int16
U8 = mybir.dt.uint8
NPBF16 = ml_dtypes.bfloat16
ALU = mybir.AluOpType
ACT = mybir.ActivationFunctionType

D = 1024
P = 4
DEPTH = 5
FAN = 4
G = 1365
RG = 341                  # route groups per p (levels 0-4)
RROWS = P * RG            # 1364 packed rows per f-plane
NS = 11                   # 128-row subtiles per f-plane
PLANE = NS * 128          # 1408
NRT = FAN * NS            # 44 route tiles
L5 = 1024                 # level-5 groups per p
E5 = 4224                 # gather elem: 4*1024 W row + 4 bias + 124 pad
T = 512
KC = D // 128             # 8
NCORES = 8
PLAT = [0, 1, 5, 21, 85, 341]

ROUTE_MODE = "hilo3"      # "hilo3" (3-pass bf16) or "f32r" (1-pass fp32r)
DEBUG = False


def build_nc(route_mode=ROUTE_MODE, debug=DEBUG):
    nc = bacc.Bacc("TRN2", target_bir_lowering=False, debug=False,
                   num_devices=NCORES)

    if route_mode == "f32r":
        xTd = nc.dram_tensor("xT", [D, T], F32R, kind="ExternalInput").ap()
        Wrd = nc.dram_tensor("Wr", [D, NRT * 128], F32R,
                             kind="ExternalInput").ap()
    else:
        xhid = nc.dram_tensor("xhi", [D, T], BF16, kind="ExternalInput").ap()
        xlod = nc.dram_tensor("xlo", [D, T], BF16, kind="ExternalInput").ap()
        Whid = nc.dram_tensor("Whi", [D, NRT * 128], BF16,
                              kind="ExternalInput").ap()
        Wlod = nc.dram_tensor("Wlo", [D, NRT * 128], BF16,
                              kind="ExternalInput").ap()
    xtokd = nc.dram_tensor("xtok", [T, D], BF16, kind="ExternalInput").ap()
    bvecd = nc.dram_tensor("bvec", [128, NRT], F32, kind="ExternalInput").ap()
    WoTd = nc.dram_tensor("WoT", [NRT * 128, D], BF16,
                          kind="ExternalInput").ap()
    W5ind = nc.dram_tensor("W5in", [P * L5, E5], BF16,
                           kind="ExternalInput").ap()
    W5od = nc.dram_tensor("W5o", [P * L5, E5], BF16,
                          kind="ExternalInput").ap()
    identd = nc.dram_tensor("ident", [128, 128], BF16,
                            kind="ExternalInput").ap()
    iotad = nc.dram_tensor("iota", [128, RG], F32, kind="ExternalInput").ap()
    outd = nc.dram_tensor("outT", [T, D], F32, kind="ExternalOutput").ap()
    if debug:
        didx = nc.dram_tensor("didx", [128, 16], F32,
                              kind="ExternalOutput").ap()
        dmask = nc.dram_tensor("dmask", [128, NS, T], BF16,
                               kind="ExternalOutput").ap()

    with tile.TileContext(nc) as tc, ExitStack() as top:
        perm = top.enter_context(tc.tile_pool(name="perm", bufs=1))
        dramp = top.enter_context(tc.tile_pool(name="dram", bufs=1,
                                               space="DRAM"))

        nc.gpsimd.load_library(MLP_LIB)

        acr = perm.tile([128, NRT, T], BF16)       # route acts (output path)
        bt = perm.tile([128, NRT], F32)
        idt = perm.tile([128, 128], BF16)
        iot = perm.tile([128, RG], F32)
        xtok = perm.tile([128, 4, D], BF16)
        mask_g = perm.tile([128, NS, T], BF16)     # group-major L0-4 mask
        prodt = perm.tile([128, D], BF16)          # dot/accum staging
        idxf = perm.tile([128, 16], F32)
        idxi = perm.tile([128, 16], I16)
        idxt = perm.tile([128, 128], I16)
        red5 = perm.tile([128, 64], F32)           # (p,c,f) dot accums
        act5 = perm.tile([128, 64], F32)           # gelu'd level-5 acts
        out5 = perm.tile([128, 4, D], BF16)        # gather contribution
        dscr = dramp.tile([16, 128], I16, name="dscr")

        nc.sync.dma_start(bt[:], bvecd[:])
        nc.sync.dma_start(idt[:], identd[:])
        nc.sync.dma_start(iot[:], iotad[:])
        nc.sync.dma_start(
            xtok[:], xtokd.rearrange("(c p) d -> p c d", p=128))
        # junk rows 84..127 of the last mask_g subtile never get written by
        # the (84-col) transpose evac; zero them once so 0*garbage != NaN
        nc.gpsimd.memset(mask_g[:], 0.0)

        with ExitStack() as gstk:
            g5p = gstk.enter_context(tc.tile_pool(name="g5", bufs=4))
            gtile = {}

            # ============ phase A + cascades + gather issue ============
            pa = gstk.enter_context(ExitStack())
            wrp = pa.enter_context(tc.tile_pool(name="wr", bufs=2))
            a32p = pa.enter_context(tc.tile_pool(name="a32", bufs=4))
            tmpp = pa.enter_context(tc.tile_pool(name="tmp", bufs=1))
            decp = pa.enter_context(tc.tile_pool(name="decs", bufs=2))
            cascp = pa.enter_context(tc.tile_pool(name="casc", bufs=1))
            psA = pa.enter_context(tc.tile_pool(name="psA", bufs=5,
                                                space="PSUM"))
            psT = pa.enter_context(tc.tile_pool(name="psT", bufs=3,
                                                space="PSUM"))

            dec1t = cascp.tile([128, 4, RROWS], BF16)  # token-major dec + 1
            mask_tok = cascp.tile([128, 4, RROWS], BF16)
            curt = cascp.tile([128, P * 4 * (DEPTH + 1)], F32)
            macc = cascp.tile([128, P * 4 * DEPTH], F32)
            nc.gpsimd.memset(mask_tok[:], 0.0)

            if route_mode == "f32r":
                xsb = cascp.tile([128, KC, T], F32R)
                for c in range(KC):
                    nc.sync.dma_start(xsb[:, c, :],
                                      xTd[c * 128:(c + 1) * 128, :])
            else:
                xh = cascp.tile([128, KC, T], BF16)
                xl = cascp.tile([128, KC, T], BF16)
                for c in range(KC):
                    nc.sync.dma_start(xh[:, c, :],
                                      xhid[c * 128:(c + 1) * 128, :])
                    nc.sync.dma_start(xl[:, c, :],
                                      xlod[c * 128:(c + 1) * 128, :])

            dec_tiles = {}
            quad = {}

            def emit_dec(s):
                a0, a1, a2, a3 = (quad.pop(f) for f in range(4))
                m01 = tmpp.tile([128, T], F32, tag="m01")
                m23 = tmpp.tile([128, T], F32, tag="m23")
                nc.vector.tensor_tensor(m01[:], a0[:], a1[:], ALU.max)
                nc.vector.tensor_tensor(m23[:], a2[:], a3[:], ALU.max)
                b1t = tmpp.tile([128, T], U8, tag="b1")
                nc.vector.tensor_tensor(b1t[:], m23[:], m01[:], ALU.is_gt)
                b01 = tmpp.tile([128, T], BF16, tag="b01")
                nc.vector.tensor_tensor(b01[:], a1[:], a0[:], ALU.is_gt)
                b23 = tmpp.tile([128, T], F32, tag="m23")
                nc.vector.tensor_tensor(b23[:], a3[:], a2[:], ALU.is_gt)
                nc.vector.select(m01[:], b1t[:], b23[:], b01[:])
                ds = dec_tiles[s]
                nc.vector.scalar_tensor_tensor(
                    ds[:], b1t[:], 2.0, m01[:], op0=ALU.mult, op1=ALU.add)

            def emit_route(f, s):
                t_lin = f * NS + s
                if f == 0:
                    dec_tiles[s] = decp.tile([128, T], BF16, tag="dec",
                                             name=f"dec_{s}")
                if route_mode == "f32r":
                    wr = wrp.tile([128, KC, 128], F32R, tag="wr",
                                  name=f"wr_{t_lin}")
                    nc.sync.dma_start(
                        wr[:],
                        Wrd[:, t_lin * 128:(t_lin + 1) * 128]
                        .rearrange("(c p) h -> p c h", p=128))
                    ps = psA.tile([128, T], F32, tag="ps", name=f"ps_{t_lin}")
                    for c in range(KC):
                        nc.tensor.matmul(ps[:], wr[:, c, :], xsb[:, c, :],
                                         start=(c == 0), stop=(c == KC - 1))
                else:
                    wh = wrp.tile([128, KC, 128], BF16, tag="wh",
                                  name=f"wh_{t_lin}")
                    wl = wrp.tile([128, KC, 128], BF16, tag="wl",
                                  name=f"wl_{t_lin}")
                    nc.sync.dma_start(
                        wh[:],
                        Whid[:, t_lin * 128:(t_lin + 1) * 128]
                        .rearrange("(c p) h -> p c h", p=128))
                    nc.sync.dma_start(
                        wl[:],
                        Wlod[:, t_lin * 128:(t_lin + 1) * 128]
                        .rearrange("(c p) h -> p c h", p=128))
                    ps = psA.tile([128, T], F32, tag="ps", name=f"ps_{t_lin}")
                    for c in range(KC):
                        nc.tensor.matmul(ps[:], wh[:, c, :], xh[:, c, :],
                                         start=(c == 0), stop=False)
                    for c in range(KC):
                        nc.tensor.matmul(ps[:], wl[:, c, :], xh[:, c, :],
                                         start=False, stop=False)
                    for c in range(KC):
                        nc.tensor.matmul(ps[:], wh[:, c, :], xl[:, c, :],
                                         start=False, stop=(c == KC - 1))
                a = a32p.tile([128, T], F32, tag="a", name=f"a_{t_lin}")
                nc.scalar.activation(a[:], ps[:], ACT.Gelu,
                                     bias=bt[:, t_lin:t_lin + 1], scale=1.0)
                nc.scalar.activation(acr[:, t_lin, :], ps[:], ACT.Gelu,
                                     bias=bt[:, t_lin:t_lin + 1], scale=1.0)
                quad[f] = a
                if f == FAN - 1:
                    emit_dec(s)

            def emit_ttrans(s):
                # dec transpose -> token-major dec+1 (scalar Copy, bias=1)
                ncol = min(128, RROWS - s * 128)
                for c in range(4):
                    pt = psT.tile([128, 128], BF16, tag="pt",
                                  name=f"ptd_{s}_{c}")
                    nc.tensor.transpose(
                        pt[:], dec_tiles[s][:, c * 128:(c + 1) * 128], idt[:])
                    nc.scalar.activation(
                        dec1t[:, c, s * 128:s * 128 + ncol], pt[:, :ncol],
                        ACT.Copy, bias=1.0)

            def emit_casc(p):
                base = p * RG
                for c in range(4):
                    pc6 = (p * 4 + c) * (DEPTH + 1)
                    pc5 = (p * 4 + c) * DEPTH
                    nc.vector.memset(mask_tok[:, c, base:base + 1], 1.0)
                    nc.vector.tensor_copy(curt[:, pc6 + 1:pc6 + 2],
                                          dec1t[:, c, base:base + 1])
                    for d in range(1, DEPTH):
                        lo, hi = PLAT[d], PLAT[d + 1]
                        n = hi - lo
                        cur = curt[:, pc6 + d:pc6 + d + 1]
                        nxt = curt[:, pc6 + d + 1:pc6 + d + 2]
                        mslice = mask_tok[:, c, base + lo:base + hi]
                        nc.vector.tensor_scalar(
                            mslice, iot[:, lo:hi], cur, None, op0=ALU.is_equal)
                        nc.vector.scalar_tensor_tensor(
                            prodt[:, :n], mslice, 1.0,
                            dec1t[:, c, base + lo:base + hi],
                            op0=ALU.mult, op1=ALU.mult,
                            accum_out=macc[:, pc5 + d:pc5 + d + 1])
                        nc.vector.scalar_tensor_tensor(
                            nxt, cur, 4.0, macc[:, pc5 + d:pc5 + d + 1],
                            op0=ALU.mult, op1=ALU.add)
                    nc.vector.tensor_scalar(
                        idxf[:, p * 4 + c:p * 4 + c + 1],
                        curt[:, pc6 + DEPTH:pc6 + DEPTH + 1],
                        float(1024 * p - RG), None, op0=ALU.add)

            def emit_idx(p):
                nc.vector.tensor_copy(idxi[:, 4 * p:4 * p + 4],
                                      idxf[:, 4 * p:4 * p + 4])
                nc.sync.dma_start(
                    dscr[4 * p:4 * p + 4, :].rearrange("m t -> t m"),
                    idxi[:, 4 * p:4 * p + 4])
                for r in range(8):
                    nc.scalar.dma_start(
                        idxt[16 * r:16 * r + 16, 32 * p:32 * p + 32],
                        dscr[4 * p:4 * p + 4, :]
                        .rearrange("m (k j) -> j (m k)", k=8))

            def emit_gather(p):
                for kind, src in (("i", W5ind), ("o", W5od)):
                    for h in range(2):
                        g = g5p.tile([128, 2, E5], BF16, tag="g5",
                                     name=f"g{kind}_{p}_{h}")
                        nc.gpsimd.dma_gather(
                            g[:], src[:],
                            idxt[:, 32 * p + 16 * h:32 * p + 16 * h + 16],
                            256, 256, E5)
                        gtile[(kind, p, h)] = g

            def emit_dots(p, h):
                # act5[t, p*16+c*4+f] = gelu(sum_d W5in[sel] * x + bias)
                g = gtile[("i", p, h)]
                for hc in range(2):
                    c = 2 * h + hc
                    for f in range(FAN):
                        r0 = p * 16 + c * 4 + f
                        nc.vector.scalar_tensor_tensor(
                            prodt[:], g[:, hc, f * D:(f + 1) * D], 1.0,
                            xtok[:, c, :], op0=ALU.mult, op1=ALU.mult,
                            accum_out=red5[:, r0:r0 + 1])
                    rs = red5[:, p * 16 + c * 4:p * 16 + c * 4 + 4]
                    nc.vector.tensor_tensor(
                        rs, rs, g[:, hc, FAN * D:FAN * D + 4], ALU.add)
                    nc.scalar.activation(
                        act5[:, p * 16 + c * 4:p * 16 + c * 4 + 4], rs,
                        ACT.Gelu, bias=0.0, scale=1.0)

            def emit_accums(p, h):
                g = gtile[("o", p, h)]
                for hc in range(2):
                    c = 2 * h + hc
                    for f in range(FAN):
                        sc = act5[:, p * 16 + c * 4 + f:
                                  p * 16 + c * 4 + f + 1]
                        gsl = g[:, hc, f * D:(f + 1) * D]
                        if p == 0 and f == 0:
                            nc.vector.tensor_scalar(
                                out5[:, c, :], gsl, sc, None, op0=ALU.mult)
                        else:
                            nc.vector.scalar_tensor_tensor(
                                out5[:, c, :], gsl, sc, out5[:, c, :],
                                op0=ALU.mult, op1=ALU.add)

            # -------- the interleaved schedule --------
            def casc_issue(p):
                emit_casc(p)
                emit_idx(p)
                emit_gather(p)

            SCHED = {4: [lambda: casc_issue(0)],
                     5: [lambda: emit_dots(0, 0)],
                     6: [lambda: emit_dots(0, 1), lambda: emit_accums(0, 0)],
                     7: [lambda: emit_accums(0, 1), lambda: casc_issue(1)],
                     8: [lambda: emit_dots(1, 0), lambda: emit_dots(1, 1)],
                     9: [lambda: emit_accums(1, 0), lambda: emit_accums(1, 1),
                         lambda: casc_issue(2)],
                     10: [lambda: emit_dots(2, 0), lambda: emit_dots(2, 1)]}
            for s in range(NS):
                for f in range(FAN):
                    emit_route(f, s)
                if s >= 1:
                    emit_ttrans(s - 1)
                for fn in SCHED.get(s, []):
                    fn()
            emit_ttrans(NS - 1)
            emit_accums(2, 0)
            emit_accums(2, 1)
            casc_issue(3)
            if debug:
                nc.sync.dma_start(didx[:], idxf[:])

            # mask transposes -> group-major (psT reused)
            for s in range(NS):
                ncol = min(128, RROWS - s * 128)
                for c in range(4):
                    pt = psT.tile([128, 128], BF16, tag="pt",
                                  name=f"ptm_{s}_{c}")
                    nc.tensor.transpose(
                        pt[0:ncol, :],
                        mask_tok[:, c, s * 128:s * 128 + ncol], idt[:])
                    nc.scalar.activation(
                        mask_g[0:ncol, s, c * 128:(c + 1) * 128],
                        pt[0:ncol, :], ACT.Copy, bias=0.0)
            if debug:
                nc.sync.dma_start(dmask[:], mask_g[:])

            pa.close()

            # ============ phase C + gather tail + combine ============
            with ExitStack() as pc:
                wop = pc.enter_context(tc.tile_pool(name="wo", bufs=4))
                mkp = pc.enter_context(tc.tile_pool(name="mk", bufs=6))
                outp = pc.enter_context(tc.tile_pool(name="outp", bufs=1))
                psC = pc.enter_context(tc.tile_pool(name="psC", bufs=1,
                                                    space="PSUM"))
                cps = psC.tile([128, 4, D], F32)
                i = 0
                for f in range(FAN):
                    for s0 in range(0, NS, 2):
                        nt = min(2, NS - s0)
                        t0 = f * NS + s0
                        wo = wop.tile([128, nt, D], BF16, tag="wo",
                                      name=f"wo_{t0}")
                        nc.sync.dma_start(
                            wo[:],
                            WoTd[t0 * 128:(t0 + nt) * 128, :]
                            .rearrange("(u p) d -> p u d", p=128))
                        for u in range(nt):
                            s = s0 + u
                            mk = mkp.tile([128, T], BF16, tag="mk",
                                          name=f"mk_{t0 + u}")
                            nc.vector.tensor_tensor(
                                mk[:], acr[:, f * NS + s, :],
                                mask_g[:, s, :], ALU.mult)
                            for c in range(4):
                                for hh in range(2):
                                    nc.tensor.matmul(
                                        cps[:, c, hh * 512:(hh + 1) * 512],
                                        mk[:, c * 128:(c + 1) * 128],
                                        wo[:, u, hh * 512:(hh + 1) * 512],
                                        start=(i == 0), stop=(i == NRT - 1))
                            i += 1

                emit_dots(3, 0)
                emit_dots(3, 1)
                emit_accums(3, 0)
                emit_accums(3, 1)

                osb = outp.tile([128, 4, D], F32)
                for c in range(4):
                    nc.vector.tensor_tensor(osb[:, c, :], cps[:, c, :],
                                            out5[:, c, :], ALU.add)
                nc.sync.dma_start(outd.rearrange("(c p) d -> p c d", p=128),
                                  osb[:])

    nc.compile()
    return nc


_NC_CACHE = {}


def _get_nc(route_mode=ROUTE_MODE, debug=DEBUG):
    key = (route_mode, debug)
    if key not in _NC_CACHE:
        _NC_CACHE[key] = build_nc(route_mode, debug)
    return _NC_CACHE[key]


def _split_hi_lo(a):
    hi = a.astype(NPBF16)
    lo = (a - hi.astype(np.float32)).astype(NPBF16)
    return hi, lo


def _prep_inputs(oldx, W_in, b_in, W_out, route_mode):
    x = np.ascontiguousarray(np.asarray(oldx, np.float32).reshape(-1, D))
    Wr4 = np.asarray(W_in, np.float32).reshape(P, G, FAN, D)
    br = np.asarray(b_in, np.float32).reshape(P, G, FAN)
    Wo4 = np.asarray(W_out, np.float32).reshape(D, P, G, FAN)

    A = np.zeros((FAN, PLANE, D), np.float32)
    A[:, :RROWS] = Wr4[:, :RG].transpose(2, 0, 1, 3).reshape(FAN, RROWS, D)
    WrT = np.ascontiguousarray(A.reshape(FAN * PLANE, D).T)  # [D, 5632]

    b_dev = np.zeros((FAN, PLANE), np.float32)
    b_dev[:, :RROWS] = br[:, :RG].transpose(2, 0, 1).reshape(FAN, RROWS)
    bvec = np.ascontiguousarray(b_dev.reshape(NRT, 128).T)   # [128, 44]

    Wo_dev = np.zeros((FAN, PLANE, D), np.float32)
    Wo_dev[:, :RROWS] = (Wo4[:, :, :RG, :].transpose(3, 1, 2, 0)
                         .reshape(FAN, RROWS, D))
    WoT = np.ascontiguousarray(Wo_dev.reshape(FAN * PLANE, D)).astype(NPBF16)

    W5in = np.zeros((P * L5, E5), np.float32)
    W5in[:, :FAN * D] = Wr4[:, RG:].reshape(P * L5, FAN * D)
    W5in[:, FAN * D:FAN * D + FAN] = br[:, RG:].reshape(P * L5, FAN)
    W5in = W5in.astype(NPBF16)
    W5o = np.zeros((P * L5, E5), np.float32)
    W5o[:, :FAN * D] = (Wo4[:, :, RG:, :].transpose(1, 2, 3, 0)
                        .reshape(P * L5, FAN * D))
    W5o = W5o.astype(NPBF16)

    iota = np.tile(np.arange(RG, dtype=np.float32), (128, 1))
    ident = np.eye(128, dtype=np.float32).astype(NPBF16)
    xT = np.ascontiguousarray(x.T)                            # [D, B]

    shared = {"bvec": bvec, "WoT": WoT, "W5in": W5in, "W5o": W5o,
              "ident": ident, "iota": iota}
    in_maps = []
    for cc in range(NCORES):
        m = dict(shared)
        xs = np.ascontiguousarray(xT[:, cc * T:(cc + 1) * T])
        m["xtok"] = np.ascontiguousarray(x[cc * T:(cc + 1) * T, :]
                                         ).astype(NPBF16)
        if route_mode == "f32r":
            m["xT"] = xs
            m["Wr"] = WrT
        else:
            m["xhi"], m["xlo"] = _split_hi_lo(xs)
            m["Whi"], m["Wlo"] = _split_hi_lo(WrT)
        in_maps.append(m)
    return in_maps


_WARM = False


def run(oldx, W_in, b_in, W_out, trace=False, route_mode=ROUTE_MODE,
        debug=DEBUG):
    nc = _get_nc(route_mode, debug)
    in_maps = _prep_inputs(oldx, W_in, b_in, W_out, route_mode)
    global _WARM
    if not _WARM:
        run_bass_kernel_spmd(nc, in_maps, list(range(NCORES)), trace=False)
        _WARM = True
    res = run_bass_kernel_spmd(nc, in_maps, list(range(NCORES)), trace=trace)
    out = np.concatenate([res.results[c]["outT"] for c in range(NCORES)],
                         axis=0)
    return np.ascontiguousarray(out).reshape(np.asarray(oldx).shape), res


def kernel(oldx, W_in, b_in, W_out):
    out, _ = run(oldx, W_in, b_in, W_out, trace=False)
    return out


# revision 5
# speedup vs baseline: 1.0439x; 1.0325x over previous
"""Trainium2 Bass kernel v2.1 for nn_FFFFanout (moe_routing tree-MLP).

Contract: kernel(**inputs) takes FULL unsharded numpy inputs
  oldx  [2, 2048, 1024] f32,  W_in [21840, 1024] f32,
  b_in  [21840] f32,          W_out [1024, 21840] f32
returns [2, 2048, 1024] f32.

v2 architecture — exploit the tree sparsity instead of dense matmuls:
  * Only tree levels 0-4 (341 of 1365 groups per p, 25% of HID) are ever
    read by the routing argmax; level-5 groups (75% of HID) contribute to
    the output only for the ONE selected group per (token, p).
  * Phase A (PE): dense matmul1 + gelu over just the 44 route tiles
    (f-major planes, rows packed p*341+g). Routing needs ~2^-15 logit
    precision (measured flip tolerance): 3-pass bf16 hi/lo. (fp32r
    1-pass measured 2^-13.5 -> 9 flipped paths -> 2.3e-2 rel err: fails.)
  * Cascade (token-major): PE-transpose dec tiles to [token, group], then
    per (p, chunk) walk the tree with small DVE ops:
    onehot(cur) = is_equal(iota, cur); m = accum(onehot * (dec+1));
    cur' = 4*cur + m. Yields the L0-4 mask + the selected level-5 group
    index as a value, already token-major for the gather index table.
  * Gather (gpsimd dma_gather): fetch only the selected level-5 W_in rows
    (4x1024 + bias) and W_out columns by int16 index; fused DVE
    multiply-reduce (accum_out) / multiply-accumulate, token-major.
  * Phase C (PE): dense matmul2 over the 44 route tiles with masked acts
    stationary, accumulating out[token, D] in PSUM; add the gather
    contribution; DMA out as [T, D] (no transposes anywhere).

v2.1 scheduling: per-p cascades + gather issues interleave INTO phase A
(dec transposes lag one subtile), gather DVE consumers are placed in the
DVE stream where their data has landed, and phase C overlaps the gather
tail. DVE uses tensor_tensor (2x) + tensor_scalar (4x, accum_out) forms
instead of scalar_tensor_tensor (which has no fast mode).
"""
import sys

if "/opt/trn_rl_repo" not in sys.path:
    sys.path.insert(0, "/opt/trn_rl_repo")

from contextlib import ExitStack

import ml_dtypes
import numpy as np

import concourse.bass as bass  # noqa: F401
import concourse.mybir as mybir
import concourse.tile as tile
from concourse import bacc
from concourse.bass_utils import run_bass_kernel_spmd
from concourse.library_config import mlp as MLP_LIB


F32 = mybir.dt.float32
F32R = mybir.dt.float32r
BF16 = mybir.dt.bfloat16
I16 = mybir.dt.int16
U8 = mybir.dt.uint8
NPBF16 = ml_dtypes.bfloat16
ALU = mybir.AluOpType
ACT = mybir.ActivationFunctionType

D = 1024
P = 4
DEPTH = 5
FAN = 4
G = 1365
RG = 341                  # route groups per p (levels 0-4)
RROWS = P * RG            # 1364 packed rows per f-plane
NS = 11                   # 128-row subtiles per f-plane
PLANE = NS * 128          # 1408
NRT = FAN * NS            # 44 route tiles
L5 = 1024                 # level-5 groups per p
E5 = 4224                 # gather elem: 4*1024 W row + 4 bias + 124 pad
T = 512
KC = D // 128             # 8
NCORES = 8
PLAT = [0, 1, 5, 21, 85, 341]

ROUTE_MODE = "hilo3"      # "hilo3" (3-pass bf16) or "f32r" (1-pass fp32r)
DEBUG = False


def build_nc(route_mode=ROUTE_MODE, debug=DEBUG):
    nc = bacc.Bacc("TRN2", target_bir_lowering=False, debug=False,
                   num_devices=NCORES)

    if route_mode == "f32r":
        xTd = nc.dram_tensor("xT", [D, T], F32R, kind="ExternalInput").ap()
        Wrd = nc.dram_tensor("Wr", [D, NRT * 128], F32R,
                             kind="ExternalInput").ap()
    else:
        xhid = nc.dram_tensor("xhi", [D, T], BF16, kind="ExternalInput").ap()
        xlod = nc.dram_tensor("xlo", [D, T], BF16, kind="ExternalInput").ap()
        Whid = nc.dram_tensor("Whi", [D, NRT * 128], BF16,
                              kind="ExternalInput").ap()
        Wlod = nc.dram_tensor("Wlo", [D, NRT * 128], BF16,
                              kind="ExternalInput").ap()
    xtokd = nc.dram_tensor("xtok", [T, D], BF16, kind="ExternalInput").ap()
    bvecd = nc.dram_tensor("bvec", [128, NRT], F32, kind="ExternalInput").ap()
    WoTd = nc.dram_tensor("WoT", [NRT * 128, D], BF16,
                          kind="ExternalInput").ap()
    W5ind = nc.dram_tensor("W5in", [P * L5, E5], BF16,
                           kind="ExternalInput").ap()
    W5od = nc.dram_tensor("W5o", [P * L5, E5], BF16,
                          kind="ExternalInput").ap()
    identd = nc.dram_tensor("ident", [128, 128], BF16,
                            kind="ExternalInput").ap()
    iotad = nc.dram_tensor("iota", [128, RG], F32, kind="ExternalInput").ap()
    outd = nc.dram_tensor("outT", [T, D], F32, kind="ExternalOutput").ap()
    if debug:
        didx = nc.dram_tensor("didx", [128, 16], F32,
                              kind="ExternalOutput").ap()
        dmask = nc.dram_tensor("dmask", [128, NS, T], BF16,
                               kind="ExternalOutput").ap()

    with tile.TileContext(nc) as tc, ExitStack() as top:
        perm = top.enter_context(tc.tile_pool(name="perm", bufs=1))
        dramp = top.enter_context(tc.tile_pool(name="dram", bufs=1,
                                               space="DRAM"))

        nc.gpsimd.load_library(MLP_LIB)

        acr = perm.tile([128, NRT, T], BF16)       # route acts (output path)
        bt = perm.tile([128, NRT], F32)
        idt = perm.tile([128, 128], BF16)
        iot = perm.tile([128, RG], F32)
        xtok = perm.tile([128, 4, D], BF16)
        mask_g = perm.tile([128, NS, T], BF16)     # group-major L0-4 mask
        prodt = perm.tile([128, D], BF16)          # dot/accum staging
        idxf = perm.tile([128, 16], F32)
        idxi = perm.tile([128, 16], I16)
        idxt = perm.tile([128, 128], I16)
        red5 = perm.tile([128, 64], F32)           # (p,c,f) dot accums
        act5 = perm.tile([128, 64], F32)           # gelu'd level-5 acts
        out5 = perm.tile([128, 4, D], BF16)        # gather contribution
        dscr = dramp.tile([16, 128], I16, name="dscr")

        nc.sync.dma_start(bt[:], bvecd[:])
        nc.sync.dma_start(idt[:], identd[:])
        nc.sync.dma_start(iot[:], iotad[:])
        nc.sync.dma_start(
            xtok[:], xtokd.rearrange("(c p) d -> p c d", p=128))
        # junk rows 84..127 of the last mask_g subtile never get written by
        # the (84-col) transpose evac; zero them once so 0*garbage != NaN
        nc.gpsimd.memset(mask_g[:], 0.0)

        with ExitStack() as gstk:
            g5p = gstk.enter_context(tc.tile_pool(name="g5", bufs=4))
            gtile = {}

            # ============ phase A + cascades + gather issue ============
            pa = gstk.enter_context(ExitStack())
            wrp = pa.enter_context(tc.tile_pool(name="wr", bufs=2))
            a32p = pa.enter_context(tc.tile_pool(name="a32", bufs=4))
            tmpp = pa.enter_context(tc.tile_pool(name="tmp", bufs=1))
            decp = pa.enter_context(tc.tile_pool(name="decs", bufs=2))
            cascp = pa.enter_context(tc.tile_pool(name="casc", bufs=1))
            psA = pa.enter_context(tc.tile_pool(name="psA", bufs=5,
                                                space="PSUM"))
            psT = pa.enter_context(tc.tile_pool(name="psT", bufs=3,
                                                space="PSUM"))

            dec1t = cascp.tile([128, 4, RROWS], BF16)  # token-major dec + 1
            mask_tok = cascp.tile([128, 4, RROWS], BF16)
            curt = cascp.tile([128, P * 4 * (DEPTH + 1)], F32)
            macc = cascp.tile([128, P * 4 * DEPTH], F32)
            nc.gpsimd.memset(mask_tok[:], 0.0)

            if route_mode == "f32r":
                xsb = cascp.tile([128, KC, T], F32R)
                for c in range(KC):
                    nc.sync.dma_start(xsb[:, c, :],
                                      xTd[c * 128:(c + 1) * 128, :])
            else:
                xh = cascp.tile([128, KC, T], BF16)
                xl = cascp.tile([128, KC, T], BF16)
                for c in range(KC):
                    nc.sync.dma_start(xh[:, c, :],
                                      xhid[c * 128:(c + 1) * 128, :])
                    nc.sync.dma_start(xl[:, c, :],
                                      xlod[c * 128:(c + 1) * 128, :])

            dec_tiles = {}
            quad = {}

            def emit_dec(s):
                a0, a1, a2, a3 = (quad.pop(f) for f in range(4))
                m01 = tmpp.tile([128, T], F32, tag="m01")
                m23 = tmpp.tile([128, T], F32, tag="m23")
                nc.vector.tensor_tensor(m01[:], a0[:], a1[:], ALU.max)
                nc.vector.tensor_tensor(m23[:], a2[:], a3[:], ALU.max)
                b1t = tmpp.tile([128, T], U8, tag="b1")
                nc.vector.tensor_tensor(b1t[:], m23[:], m01[:], ALU.is_gt)
                b01 = tmpp.tile([128, T], BF16, tag="b01")
                nc.vector.tensor_tensor(b01[:], a1[:], a0[:], ALU.is_gt)
                b23 = tmpp.tile([128, T], F32, tag="m23")
                nc.vector.tensor_tensor(b23[:], a3[:], a2[:], ALU.is_gt)
                nc.vector.select(m01[:], b1t[:], b23[:], b01[:])
                ds = dec_tiles[s]
                nc.vector.scalar_tensor_tensor(
                    ds[:], b1t[:], 2.0, m01[:], op0=ALU.mult, op1=ALU.add)

            def emit_route(f, s):
                t_lin = f * NS + s
                if f == 0:
                    dec_tiles[s] = decp.tile([128, T], BF16, tag="dec",
                                             name=f"dec_{s}")
                if route_mode == "f32r":
                    wr = wrp.tile([128, KC, 128], F32R, tag="wr",
                                  name=f"wr_{t_lin}")
                    nc.sync.dma_start(
                        wr[:],
                        Wrd[:, t_lin * 128:(t_lin + 1) * 128]
                        .rearrange("(c p) h -> p c h", p=128))
                    ps = psA.tile([128, T], F32, tag="ps", name=f"ps_{t_lin}")
                    for c in range(KC):
                        nc.tensor.matmul(ps[:], wr[:, c, :], xsb[:, c, :],
                                         start=(c == 0), stop=(c == KC - 1))
                else:
                    wh = wrp.tile([128, KC, 128], BF16, tag="wh",
                                  name=f"wh_{t_lin}")
                    wl = wrp.tile([128, KC, 128], BF16, tag="wl",
                                  name=f"wl_{t_lin}")
                    nc.sync.dma_start(
                        wh[:],
                        Whid[:, t_lin * 128:(t_lin + 1) * 128]
                        .rearrange("(c p) h -> p c h", p=128))
                    nc.sync.dma_start(
                        wl[:],
                        Wlod[:, t_lin * 128:(t_lin + 1) * 128]
                        .rearrange("(c p) h -> p c h", p=128))
                    ps = psA.tile([128, T], F32, tag="ps", name=f"ps_{t_lin}")
                    for c in range(KC):
                        nc.tensor.matmul(ps[:], wh[:, c, :], xh[:, c, :],
                                         start=(c == 0), stop=False)
                    for c in range(KC):
                        nc.tensor.matmul(ps[:], wl[:, c, :], xh[:, c, :],
                                         start=False, stop=False)
                    for c in range(KC):
                        nc.tensor.matmul(ps[:], wh[:, c, :], xl[:, c, :],
                                         start=False, stop=(c == KC - 1))
                a = a32p.tile([128, T], F32, tag="a", name=f"a_{t_lin}")
                nc.scalar.activation(a[:], ps[:], ACT.Gelu,
                                     bias=bt[:, t_lin:t_lin + 1], scale=1.0)
                nc.scalar.activation(acr[:, t_lin, :], ps[:], ACT.Gelu,
                                     bias=bt[:, t_lin:t_lin + 1], scale=1.0)
                quad[f] = a
                if f == FAN - 1:
                    emit_dec(s)

            def emit_ttrans(s):
                # dec transpose -> token-major dec+1 (scalar Copy, bias=1)
                ncol = min(128, RROWS - s * 128)
                for c in range(4):
                    pt = psT.tile([128, 128], BF16, tag="pt",
                                  name=f"ptd_{s}_{c}")
                    nc.tensor.transpose(
                        pt[:], dec_tiles[s][:, c * 128:(c + 1) * 128], idt[:])
                    nc.scalar.activation(
                        dec1t[:, c, s * 128:s * 128 + ncol], pt[:, :ncol],
                        ACT.Copy, bias=1.0)

            def emit_casc(p):
                base = p * RG
                for c in range(4):
                    pc6 = (p * 4 + c) * (DEPTH + 1)
                    pc5 = (p * 4 + c) * DEPTH
                    nc.vector.memset(mask_tok[:, c, base:base + 1], 1.0)
                    nc.vector.tensor_copy(curt[:, pc6 + 1:pc6 + 2],
                                          dec1t[:, c, base:base + 1])
                    for d in range(1, DEPTH):
                        lo, hi = PLAT[d], PLAT[d + 1]
                        n = hi - lo
                        cur = curt[:, pc6 + d:pc6 + d + 1]
                        nxt = curt[:, pc6 + d + 1:pc6 + d + 2]
                        mslice = mask_tok[:, c, base + lo:base + hi]
                        nc.vector.tensor_scalar(
                            mslice, iot[:, lo:hi], cur, None, op0=ALU.is_equal)
                        nc.vector.scalar_tensor_tensor(
                            prodt[:, :n], mslice, 1.0,
                            dec1t[:, c, base + lo:base + hi],
                            op0=ALU.mult, op1=ALU.mult,
                            accum_out=macc[:, pc5 + d:pc5 + d + 1])
                        nc.vector.scalar_tensor_tensor(
                            nxt, cur, 4.0, macc[:, pc5 + d:pc5 + d + 1],
                            op0=ALU.mult, op1=ALU.add)
                    nc.vector.tensor_scalar(
                        idxf[:, p * 4 + c:p * 4 + c + 1],
                        curt[:, pc6 + DEPTH:pc6 + DEPTH + 1],
                        float(1024 * p - RG), None, op0=ALU.add)

            def emit_idx(p):
                nc.vector.tensor_copy(idxi[:, 4 * p:4 * p + 4],
                                      idxf[:, 4 * p:4 * p + 4])
                nc.sync.dma_start(
                    dscr[4 * p:4 * p + 4, :].rearrange("m t -> t m"),
                    idxi[:, 4 * p:4 * p + 4])
                for r in range(8):
                    nc.scalar.dma_start(
                        idxt[16 * r:16 * r + 16, 32 * p:32 * p + 32],
                        dscr[4 * p:4 * p + 4, :]
                        .rearrange("m (k j) -> j (m k)", k=8))

            def emit_gather(p):
                for kind, src in (("i", W5ind), ("o", W5od)):
                    for h in range(2):
                        g = g5p.tile([128, 2, E5], BF16, tag="g5",
                                     name=f"g{kind}_{p}_{h}")
                        nc.gpsimd.dma_gather(
                            g[:], src[:],
                            idxt[:, 32 * p + 16 * h:32 * p + 16 * h + 16],
                            256, 256, E5)
                        gtile[(kind, p, h)] = g

            def emit_dots(p, h):
                # act5[t, p*16+c*4+f] = gelu(sum_d W5in[sel] * x + bias)
                g = gtile[("i", p, h)]
                for hc in range(2):
                    c = 2 * h + hc
                    for f in range(FAN):
                        r0 = p * 16 + c * 4 + f
                        nc.vector.scalar_tensor_tensor(
                            prodt[:], g[:, hc, f * D:(f + 1) * D], 1.0,
                            xtok[:, c, :], op0=ALU.mult, op1=ALU.mult,
                            accum_out=red5[:, r0:r0 + 1])
                    rs = red5[:, p * 16 + c * 4:p * 16 + c * 4 + 4]
                    nc.vector.tensor_tensor(
                        rs, rs, g[:, hc, FAN * D:FAN * D + 4], ALU.add)
                    nc.scalar.activation(
                        act5[:, p * 16 + c * 4:p * 16 + c * 4 + 4], rs,
                        ACT.Gelu, bias=0.0, scale=1.0)

            def emit_accums(p, h):
                g = gtile[("o", p, h)]
                for hc in range(2):
                    c = 2 * h + hc
                    for f in range(FAN):
                        sc = act5[:, p * 16 + c * 4 + f:
                                  p * 16 + c * 4 + f + 1]
                        gsl = g[:, hc, f * D:(f + 1) * D]
                        if p == 0 and f == 0:
                            nc.vector.tensor_scalar(
                                out5[:, c, :], gsl, sc, None, op0=ALU.mult)
                        else:
                            nc.vector.scalar_tensor_tensor(
                                out5[:, c, :], gsl, sc, out5[:, c, :],
                                op0=ALU.mult, op1=ALU.add)

            # -------- the interleaved schedule --------
            def casc_issue(p):
                emit_casc(p)
                emit_idx(p)
                emit_gather(p)

            SCHED = {4: [lambda: casc_issue(0)],
                     5: [lambda: emit_dots(0, 0)],
                     6: [lambda: emit_dots(0, 1), lambda: emit_accums(0, 0)],
                     7: [lambda: emit_accums(0, 1), lambda: casc_issue(1)],
                     8: [lambda: emit_dots(1, 0), lambda: emit_dots(1, 1)],
                     9: [lambda: emit_accums(1, 0), lambda: emit_accums(1, 1),
                         lambda: casc_issue(2)],
                     10: []}
            for s in range(NS):
                for f in range(FAN):
                    emit_route(f, s)
                if s >= 1:
                    emit_ttrans(s - 1)
                for fn in SCHED.get(s, []):
                    fn()
            emit_ttrans(NS - 1)
            casc_issue(3)
            if debug:
                nc.sync.dma_start(didx[:], idxf[:])

            # mask transposes -> group-major (psT reused)
            for s in range(NS):
                ncol = min(128, RROWS - s * 128)
                for c in range(4):
                    pt = psT.tile([128, 128], BF16, tag="pt",
                                  name=f"ptm_{s}_{c}")
                    nc.tensor.transpose(
                        pt[0:ncol, :],
                        mask_tok[:, c, s * 128:s * 128 + ncol], idt[:])
                    nc.scalar.activation(
                        mask_g[0:ncol, s, c * 128:(c + 1) * 128],
                        pt[0:ncol, :], ACT.Copy, bias=0.0)
            if debug:
                nc.sync.dma_start(dmask[:], mask_g[:])

            pa.close()

            # ============ phase C + gather tail + combine ============
            with ExitStack() as pc:
                wop = pc.enter_context(tc.tile_pool(name="wo", bufs=4))
                mkp = pc.enter_context(tc.tile_pool(name="mk", bufs=6))
                outp = pc.enter_context(tc.tile_pool(name="outp", bufs=1))
                psC = pc.enter_context(tc.tile_pool(name="psC", bufs=1,
                                                    space="PSUM"))
                cps = psC.tile([128, 4, D], F32)
                i = 0
                for f in range(FAN):
                    for s0 in range(0, NS, 2):
                        nt = min(2, NS - s0)
                        t0 = f * NS + s0
                        wo = wop.tile([128, nt, D], BF16, tag="wo",
                                      name=f"wo_{t0}")
                        nc.sync.dma_start(
                            wo[:],
                            WoTd[t0 * 128:(t0 + nt) * 128, :]
                            .rearrange("(u p) d -> p u d", p=128))
                        for u in range(nt):
                            s = s0 + u
                            mk = mkp.tile([128, T], BF16, tag="mk",
                                          name=f"mk_{t0 + u}")
                            nc.vector.tensor_tensor(
                                mk[:], acr[:, f * NS + s, :],
                                mask_g[:, s, :], ALU.mult)
                            for c in range(4):
                                for hh in range(2):
                                    nc.tensor.matmul(
                                        cps[:, c, hh * 512:(hh + 1) * 512],
                                        mk[:, c * 128:(c + 1) * 128],
                                        wo[:, u, hh * 512:(hh + 1) * 512],
                                        start=(i == 0), stop=(i == NRT - 1))
                            i += 1

                emit_dots(2, 0)
                emit_dots(2, 1)
                emit_accums(2, 0)
                emit_accums(2, 1)
                emit_dots(3, 0)
                emit_dots(3, 1)
                emit_accums(3, 0)
                emit_accums(3, 1)

                osb = outp.tile([128, 4, D], F32)
                for c in range(4):
                    nc.vector.tensor_tensor(osb[:, c, :], cps[:, c, :],
                                            out5[:, c, :], ALU.add)
                nc.sync.dma_start(outd.rearrange("(c p) d -> p c d", p=128),
                                  osb[:])

    nc.compile()
    return nc


_NC_CACHE = {}


def _get_nc(route_mode=ROUTE_MODE, debug=DEBUG):
    key = (route_mode, debug)
    if key not in _NC_CACHE:
        _NC_CACHE[key] = build_nc(route_mode, debug)
    return _NC_CACHE[key]


def _split_hi_lo(a):
    hi = a.astype(NPBF16)
    lo = (a - hi.astype(np.float32)).astype(NPBF16)
    return hi, lo


def _prep_inputs(oldx, W_in, b_in, W_out, route_mode):
    x = np.ascontiguousarray(np.asarray(oldx, np.float32).reshape(-1, D))
    Wr4 = np.asarray(W_in, np.float32).reshape(P, G, FAN, D)
    br = np.asarray(b_in, np.float32).reshape(P, G, FAN)
    Wo4 = np.asarray(W_out, np.float32).reshape(D, P, G, FAN)

    A = np.zeros((FAN, PLANE, D), np.float32)
    A[:, :RROWS] = Wr4[:, :RG].transpose(2, 0, 1, 3).reshape(FAN, RROWS, D)
    WrT = np.ascontiguousarray(A.reshape(FAN * PLANE, D).T)  # [D, 5632]

    b_dev = np.zeros((FAN, PLANE), np.float32)
    b_dev[:, :RROWS] = br[:, :RG].transpose(2, 0, 1).reshape(FAN, RROWS)
    bvec = np.ascontiguousarray(b_dev.reshape(NRT, 128).T)   # [128, 44]

    Wo_dev = np.zeros((FAN, PLANE, D), np.float32)
    Wo_dev[:, :RROWS] = (Wo4[:, :, :RG, :].transpose(3, 1, 2, 0)
                         .reshape(FAN, RROWS, D))
    WoT = np.ascontiguousarray(Wo_dev.reshape(FAN * PLANE, D)).astype(NPBF16)

    W5in = np.zeros((P * L5, E5), np.float32)
    W5in[:, :FAN * D] = Wr4[:, RG:].reshape(P * L5, FAN * D)
    W5in[:, FAN * D:FAN * D + FAN] = br[:, RG:].reshape(P * L5, FAN)
    W5in = W5in.astype(NPBF16)
    W5o = np.zeros((P * L5, E5), np.float32)
    W5o[:, :FAN * D] = (Wo4[:, :, RG:, :].transpose(1, 2, 3, 0)
                        .reshape(P * L5, FAN * D))
    W5o = W5o.astype(NPBF16)

    iota = np.tile(np.arange(RG, dtype=np.float32), (128, 1))
    ident = np.eye(128, dtype=np.float32).astype(NPBF16)
    xT = np.ascontiguousarray(x.T)                            # [D, B]

    shared = {"bvec": bvec, "WoT": WoT, "W5in": W5in, "W5o": W5o,
              "ident": ident, "iota": iota}
    in_maps = []
    for cc in range(NCORES):
        m = dict(shared)
        xs = np.ascontiguousarray(xT[:, cc * T:(cc + 1) * T])
        m["xtok"] = np.ascontiguousarray(x[cc * T:(cc + 1) * T, :]
                                         ).astype(NPBF16)
        if route_mode == "f32r":
            m["xT"] = xs
            m["Wr"] = WrT
        else:
            m["xhi"], m["xlo"] = _split_hi_lo(xs)
            m["Whi"], m["Wlo"] = _split_hi_lo(WrT)
        in_maps.append(m)
    return in_maps


_WARM = False


def run(oldx, W_in, b_in, W_out, trace=False, route_mode=ROUTE_MODE,
        debug=DEBUG):
    nc = _get_nc(route_mode, debug)
    in_maps = _prep_inputs(oldx, W_in, b_in, W_out, route_mode)
    global _WARM
    if not _WARM:
        run_bass_kernel_spmd(nc, in_maps, list(range(NCORES)), trace=False)
        _WARM = True
    res = run_bass_kernel_spmd(nc, in_maps, list(range(NCORES)), trace=trace)
    out = np.concatenate([res.results[c]["outT"] for c in range(NCORES)],
                         axis=0)
    return np.ascontiguousarray(out).reshape(np.asarray(oldx).shape), res


def kernel(oldx, W_in, b_in, W_out):
    out, _ = run(oldx, W_in, b_in, W_out, trace=False)
    return out


# revision 6
# speedup vs baseline: 1.0631x; 1.0184x over previous
"""Trainium2 Bass kernel v2.1 for nn_FFFFanout (moe_routing tree-MLP).

Contract: kernel(**inputs) takes FULL unsharded numpy inputs
  oldx  [2, 2048, 1024] f32,  W_in [21840, 1024] f32,
  b_in  [21840] f32,          W_out [1024, 21840] f32
returns [2, 2048, 1024] f32.

v2 architecture — exploit the tree sparsity instead of dense matmuls:
  * Only tree levels 0-4 (341 of 1365 groups per p, 25% of HID) are ever
    read by the routing argmax; level-5 groups (75% of HID) contribute to
    the output only for the ONE selected group per (token, p).
  * Phase A (PE): dense matmul1 + gelu over just the 44 route tiles
    (f-major planes, rows packed p*341+g). Routing needs ~2^-15 logit
    precision (measured flip tolerance): 3-pass bf16 hi/lo. (fp32r
    1-pass measured 2^-13.5 -> 9 flipped paths -> 2.3e-2 rel err: fails.)
  * Cascade (token-major): PE-transpose dec tiles to [token, group], then
    per (p, chunk) walk the tree with small DVE ops:
    onehot(cur) = is_equal(iota, cur); m = accum(onehot * (dec+1));
    cur' = 4*cur + m. Yields the L0-4 mask + the selected level-5 group
    index as a value, already token-major for the gather index table.
  * Gather (gpsimd dma_gather): fetch only the selected level-5 W_in rows
    (4x1024 + bias) and W_out columns by int16 index; fused DVE
    multiply-reduce (accum_out) / multiply-accumulate, token-major.
  * Phase C (PE): dense matmul2 over the 44 route tiles with masked acts
    stationary, accumulating out[token, D] in PSUM; add the gather
    contribution; DMA out as [T, D] (no transposes anywhere).

v2.1 scheduling: per-p cascades + gather issues interleave INTO phase A
(dec transposes lag one subtile), gather DVE consumers are placed in the
DVE stream where their data has landed, and phase C overlaps the gather
tail. DVE uses tensor_tensor (2x) + tensor_scalar (4x, accum_out) forms
instead of scalar_tensor_tensor (which has no fast mode).
"""
import sys

if "/opt/trn_rl_repo" not in sys.path:
    sys.path.insert(0, "/opt/trn_rl_repo")

from contextlib import ExitStack

import ml_dtypes
import numpy as np

import concourse.bass as bass  # noqa: F401
import concourse.mybir as mybir
import concourse.tile as tile
from concourse import bacc
from concourse.bass_utils import run_bass_kernel_spmd
from concourse.library_config import mlp as MLP_LIB


F32 = mybir.dt.float32
F32R = mybir.dt.float32r
BF16 = mybir.dt.bfloat16
I16 = mybir.dt.int16
U8 = mybir.dt.uint8
NPBF16 = ml_dtypes.bfloat16
ALU = mybir.AluOpType
ACT = mybir.ActivationFunctionType

D = 1024
P = 4
DEPTH = 5
FAN = 4
G = 1365
RG = 341                  # route groups per p (levels 0-4)
RROWS = P * RG            # 1364 packed rows per f-plane
NS = 11                   # 128-row subtiles per f-plane
PLANE = NS * 128          # 1408
NRT = FAN * NS            # 44 route tiles
L5 = 1024                 # level-5 groups per p
E5 = 4224                 # gather elem: 4*1024 W row + 4 bias + 124 pad
T = 512
KC = D // 128             # 8
NCORES = 8
PLAT = [0, 1, 5, 21, 85, 341]

ROUTE_MODE = "hilo3"      # "hilo3" (3-pass bf16) or "f32r" (1-pass fp32r)
DEBUG = False


def build_nc(route_mode=ROUTE_MODE, debug=DEBUG):
    nc = bacc.Bacc("TRN2", target_bir_lowering=False, debug=False,
                   num_devices=NCORES)

    if route_mode == "f32r":
        xTd = nc.dram_tensor("xT", [D, T], F32R, kind="ExternalInput").ap()
        Wrd = nc.dram_tensor("Wr", [D, NRT * 128], F32R,
                             kind="ExternalInput").ap()
    else:
        xhid = nc.dram_tensor("xhi", [D, T], BF16, kind="ExternalInput").ap()
        xlod = nc.dram_tensor("xlo", [D, T], BF16, kind="ExternalInput").ap()
        Whid = nc.dram_tensor("Whi", [D, NRT * 128], BF16,
                              kind="ExternalInput").ap()
        Wlod = nc.dram_tensor("Wlo", [D, NRT * 128], BF16,
                              kind="ExternalInput").ap()
    xtokd = nc.dram_tensor("xtok", [T, D], BF16, kind="ExternalInput").ap()
    bvecd = nc.dram_tensor("bvec", [128, NRT], F32, kind="ExternalInput").ap()
    WoTd = nc.dram_tensor("WoT", [NRT * 128, D], BF16,
                          kind="ExternalInput").ap()
    W5ind = nc.dram_tensor("W5in", [P * L5, E5], BF16,
                           kind="ExternalInput").ap()
    W5od = nc.dram_tensor("W5o", [P * L5, E5], BF16,
                          kind="ExternalInput").ap()
    identd = nc.dram_tensor("ident", [128, 128], BF16,
                            kind="ExternalInput").ap()
    iotad = nc.dram_tensor("iota", [128, RG], F32, kind="ExternalInput").ap()
    outd = nc.dram_tensor("outT", [T, D], F32, kind="ExternalOutput").ap()
    if debug:
        didx = nc.dram_tensor("didx", [128, 16], F32,
                              kind="ExternalOutput").ap()
        dmask = nc.dram_tensor("dmask", [128, NS, T], BF16,
                               kind="ExternalOutput").ap()

    with tile.TileContext(nc) as tc, ExitStack() as top:
        perm = top.enter_context(tc.tile_pool(name="perm", bufs=1))
        dramp = top.enter_context(tc.tile_pool(name="dram", bufs=1,
                                               space="DRAM"))

        nc.gpsimd.load_library(MLP_LIB)

        acr = perm.tile([128, NRT, T], BF16)       # route acts (output path)
        bt = perm.tile([128, NRT], F32)
        idt = perm.tile([128, 128], BF16)
        iot = perm.tile([128, RG], F32)
        xtok = perm.tile([128, 4, D], BF16)
        mask_g = perm.tile([128, NS, T], BF16)     # group-major L0-4 mask
        prodt = perm.tile([128, D], BF16)          # dot/accum staging
        idxf = perm.tile([128, 16], F32)
        idxi = perm.tile([128, 16], I16)
        idxt = perm.tile([128, 128], I16)
        red5 = perm.tile([128, 64], F32)           # (p,c,f) dot accums
        act5 = perm.tile([128, 64], F32)           # gelu'd level-5 acts
        out5 = perm.tile([128, 4, D], BF16)        # gather contribution
        dscr = dramp.tile([16, 128], I16, name="dscr")

        nc.sync.dma_start(bt[:], bvecd[:])
        nc.sync.dma_start(idt[:], identd[:])
        nc.sync.dma_start(iot[:], iotad[:])
        nc.sync.dma_start(
            xtok[:], xtokd.rearrange("(c p) d -> p c d", p=128))
        # junk rows 84..127 of the last mask_g subtile never get written by
        # the (84-col) transpose evac; zero them once so 0*garbage != NaN
        nc.gpsimd.memset(mask_g[:], 0.0)

        with ExitStack() as gstk:
            g5p = gstk.enter_context(tc.tile_pool(name="g5", bufs=4))
            gtile = {}

            # ============ phase A + cascades + gather issue ============
            pa = gstk.enter_context(ExitStack())
            wrp = pa.enter_context(tc.tile_pool(name="wr", bufs=2))
            a32p = pa.enter_context(tc.tile_pool(name="a32", bufs=4))
            tmpp = pa.enter_context(tc.tile_pool(name="tmp", bufs=1))
            decp = pa.enter_context(tc.tile_pool(name="decs", bufs=2))
            cascp = pa.enter_context(tc.tile_pool(name="casc", bufs=1))
            psA = pa.enter_context(tc.tile_pool(name="psA", bufs=6,
                                                space="PSUM"))
            psT = pa.enter_context(tc.tile_pool(name="psT", bufs=2,
                                                space="PSUM"))

            dec1t = cascp.tile([128, 4, RROWS], BF16)  # token-major dec + 1
            mask_tok = cascp.tile([128, 4, RROWS], BF16)
            curt = cascp.tile([128, P * 4 * (DEPTH + 1)], F32)
            macc = cascp.tile([128, P * 4 * DEPTH], F32)
            nc.gpsimd.memset(mask_tok[:], 0.0)

            if route_mode == "f32r":
                xsb = cascp.tile([128, KC, T], F32R)
                for c in range(KC):
                    nc.sync.dma_start(xsb[:, c, :],
                                      xTd[c * 128:(c + 1) * 128, :])
            else:
                xh = cascp.tile([128, KC, T], BF16)
                xl = cascp.tile([128, KC, T], BF16)
                for c in range(KC):
                    nc.sync.dma_start(xh[:, c, :],
                                      xhid[c * 128:(c + 1) * 128, :])
                    nc.sync.dma_start(xl[:, c, :],
                                      xlod[c * 128:(c + 1) * 128, :])

            dec_tiles = {}
            quad = {}

            def emit_dec(s):
                a0, a1, a2, a3 = (quad.pop(f) for f in range(4))
                m01 = tmpp.tile([128, T], F32, tag="m01")
                m23 = tmpp.tile([128, T], F32, tag="m23")
                nc.vector.tensor_tensor(m01[:], a0[:], a1[:], ALU.max)
                nc.vector.tensor_tensor(m23[:], a2[:], a3[:], ALU.max)
                b1t = tmpp.tile([128, T], U8, tag="b1")
                nc.vector.tensor_tensor(b1t[:], m23[:], m01[:], ALU.is_gt)
                b01 = tmpp.tile([128, T], BF16, tag="b01")
                nc.vector.tensor_tensor(b01[:], a1[:], a0[:], ALU.is_gt)
                b23 = tmpp.tile([128, T], F32, tag="m23")
                nc.vector.tensor_tensor(b23[:], a3[:], a2[:], ALU.is_gt)
                nc.vector.select(m01[:], b1t[:], b23[:], b01[:])
                ds = dec_tiles[s]
                nc.vector.scalar_tensor_tensor(
                    ds[:], b1t[:], 2.0, m01[:], op0=ALU.mult, op1=ALU.add)

            def emit_route(f, s):
                t_lin = f * NS + s
                if f == 0:
                    dec_tiles[s] = decp.tile([128, T], BF16, tag="dec",
                                             name=f"dec_{s}")
                if route_mode == "f32r":
                    wr = wrp.tile([128, KC, 128], F32R, tag="wr",
                                  name=f"wr_{t_lin}")
                    nc.sync.dma_start(
                        wr[:],
                        Wrd[:, t_lin * 128:(t_lin + 1) * 128]
                        .rearrange("(c p) h -> p c h", p=128))
                    ps = psA.tile([128, T], F32, tag="ps", name=f"ps_{t_lin}")
                    for c in range(KC):
                        nc.tensor.matmul(ps[:], wr[:, c, :], xsb[:, c, :],
                                         start=(c == 0), stop=(c == KC - 1))
                else:
                    wh = wrp.tile([128, KC, 128], BF16, tag="wh",
                                  name=f"wh_{t_lin}")
                    wl = wrp.tile([128, KC, 128], BF16, tag="wl",
                                  name=f"wl_{t_lin}")
                    nc.sync.dma_start(
                        wh[:],
                        Whid[:, t_lin * 128:(t_lin + 1) * 128]
                        .rearrange("(c p) h -> p c h", p=128))
                    nc.sync.dma_start(
                        wl[:],
                        Wlod[:, t_lin * 128:(t_lin + 1) * 128]
                        .rearrange("(c p) h -> p c h", p=128))
                    ps = psA.tile([128, T], F32, tag="ps", name=f"ps_{t_lin}")
                    for c in range(KC):
                        nc.tensor.matmul(ps[:], wh[:, c, :], xh[:, c, :],
                                         start=(c == 0), stop=False)
                    for c in range(KC):
                        nc.tensor.matmul(ps[:], wl[:, c, :], xh[:, c, :],
                                         start=False, stop=False)
                    for c in range(KC):
                        nc.tensor.matmul(ps[:], wh[:, c, :], xl[:, c, :],
                                         start=False, stop=(c == KC - 1))
                a = a32p.tile([128, T], F32, tag="a", name=f"a_{t_lin}")
                nc.scalar.activation(a[:], ps[:], ACT.Gelu,
                                     bias=bt[:, t_lin:t_lin + 1], scale=1.0)
                nc.scalar.activation(acr[:, t_lin, :], ps[:], ACT.Gelu,
                                     bias=bt[:, t_lin:t_lin + 1], scale=1.0)
                quad[f] = a
                if f == FAN - 1:
                    emit_dec(s)

            def emit_ttrans(s):
                # dec transpose -> token-major dec+1 (scalar Copy, bias=1)
                ncol = min(128, RROWS - s * 128)
                for c in range(4):
                    pt = psT.tile([128, 128], BF16, tag="pt",
                                  name=f"ptd_{s}_{c}")
                    nc.tensor.transpose(
                        pt[:], dec_tiles[s][:, c * 128:(c + 1) * 128], idt[:])
                    nc.scalar.activation(
                        dec1t[:, c, s * 128:s * 128 + ncol], pt[:, :ncol],
                        ACT.Copy, bias=1.0)

            def emit_casc(p):
                base = p * RG
                for c in range(4):
                    pc6 = (p * 4 + c) * (DEPTH + 1)
                    pc5 = (p * 4 + c) * DEPTH
                    nc.vector.memset(mask_tok[:, c, base:base + 1], 1.0)
                    nc.vector.tensor_copy(curt[:, pc6 + 1:pc6 + 2],
                                          dec1t[:, c, base:base + 1])
                    for d in range(1, DEPTH):
                        lo, hi = PLAT[d], PLAT[d + 1]
                        n = hi - lo
                        cur = curt[:, pc6 + d:pc6 + d + 1]
                        nxt = curt[:, pc6 + d + 1:pc6 + d + 2]
                        mslice = mask_tok[:, c, base + lo:base + hi]
                        nc.vector.tensor_scalar(
                            mslice, iot[:, lo:hi], cur, None, op0=ALU.is_equal)
                        nc.vector.scalar_tensor_tensor(
                            prodt[:, :n], mslice, 1.0,
                            dec1t[:, c, base + lo:base + hi],
                            op0=ALU.mult, op1=ALU.mult,
                            accum_out=macc[:, pc5 + d:pc5 + d + 1])
                        nc.vector.scalar_tensor_tensor(
                            nxt, cur, 4.0, macc[:, pc5 + d:pc5 + d + 1],
                            op0=ALU.mult, op1=ALU.add)
                    nc.vector.tensor_scalar(
                        idxf[:, p * 4 + c:p * 4 + c + 1],
                        curt[:, pc6 + DEPTH:pc6 + DEPTH + 1],
                        float(1024 * p - RG), None, op0=ALU.add)

            def emit_idx(p):
                nc.vector.tensor_copy(idxi[:, 4 * p:4 * p + 4],
                                      idxf[:, 4 * p:4 * p + 4])
                nc.sync.dma_start(
                    dscr[4 * p:4 * p + 4, :].rearrange("m t -> t m"),
                    idxi[:, 4 * p:4 * p + 4])
                for r in range(8):
                    nc.sync.dma_start(
                        idxt[16 * r:16 * r + 16, 32 * p:32 * p + 32],
                        dscr[4 * p:4 * p + 4, :]
                        .rearrange("m (k j) -> j (m k)", k=8))

            def emit_gather(p):
                for kind, src in (("i", W5ind), ("o", W5od)):
                    for h in range(2):
                        g = g5p.tile([128, 2, E5], BF16, tag="g5",
                                     name=f"g{kind}_{p}_{h}")
                        nc.gpsimd.dma_gather(
                            g[:], src[:],
                            idxt[:, 32 * p + 16 * h:32 * p + 16 * h + 16],
                            256, 256, E5)
                        gtile[(kind, p, h)] = g

            def emit_dots(p, h):
                # act5[t, p*16+c*4+f] = gelu(sum_d W5in[sel] * x + bias)
                g = gtile[("i", p, h)]
                for hc in range(2):
                    c = 2 * h + hc
                    for f in range(FAN):
                        r0 = p * 16 + c * 4 + f
                        nc.vector.scalar_tensor_tensor(
                            prodt[:], g[:, hc, f * D:(f + 1) * D], 1.0,
                            xtok[:, c, :], op0=ALU.mult, op1=ALU.mult,
                            accum_out=red5[:, r0:r0 + 1])
                    rs = red5[:, p * 16 + c * 4:p * 16 + c * 4 + 4]
                    nc.vector.tensor_tensor(
                        rs, rs, g[:, hc, FAN * D:FAN * D + 4], ALU.add)
                    nc.scalar.activation(
                        act5[:, p * 16 + c * 4:p * 16 + c * 4 + 4], rs,
                        ACT.Gelu, bias=0.0, scale=1.0)

            def emit_accums(p, h):
                g = gtile[("o", p, h)]
                for hc in range(2):
                    c = 2 * h + hc
                    for f in range(FAN):
                        sc = act5[:, p * 16 + c * 4 + f:
                                  p * 16 + c * 4 + f + 1]
                        gsl = g[:, hc, f * D:(f + 1) * D]
                        if p == 0 and f == 0:
                            nc.vector.tensor_scalar(
                                out5[:, c, :], gsl, sc, None, op0=ALU.mult)
                        else:
                            nc.vector.scalar_tensor_tensor(
                                out5[:, c, :], gsl, sc, out5[:, c, :],
                                op0=ALU.mult, op1=ALU.add)

            # -------- the interleaved schedule --------
            def casc_issue(p):
                emit_casc(p)
                emit_idx(p)
                emit_gather(p)

            SCHED = {4: [lambda: casc_issue(0)],
                     5: [lambda: emit_dots(0, 0)],
                     6: [lambda: emit_dots(0, 1), lambda: emit_accums(0, 0)],
                     7: [lambda: emit_accums(0, 1), lambda: casc_issue(1)],
                     8: [lambda: emit_dots(1, 0), lambda: emit_dots(1, 1)],
                     9: [lambda: emit_accums(1, 0), lambda: emit_accums(1, 1),
                         lambda: casc_issue(2)],
                     10: []}
            for s in range(NS):
                for f in range(FAN):
                    emit_route(f, s)
                if s >= 1:
                    emit_ttrans(s - 1)
                for fn in SCHED.get(s, []):
                    fn()
            emit_ttrans(NS - 1)
            casc_issue(3)
            if debug:
                nc.sync.dma_start(didx[:], idxf[:])

            # mask transposes -> group-major (psT reused)
            for s in range(NS):
                ncol = min(128, RROWS - s * 128)
                for c in range(4):
                    pt = psT.tile([128, 128], BF16, tag="pt",
                                  name=f"ptm_{s}_{c}")
                    nc.tensor.transpose(
                        pt[0:ncol, :],
                        mask_tok[:, c, s * 128:s * 128 + ncol], idt[:])
                    nc.scalar.activation(
                        mask_g[0:ncol, s, c * 128:(c + 1) * 128],
                        pt[0:ncol, :], ACT.Copy, bias=0.0)
            if debug:
                nc.sync.dma_start(dmask[:], mask_g[:])

            pa.close()

            # ============ phase C + gather tail + combine ============
            with ExitStack() as pc:
                wop = pc.enter_context(tc.tile_pool(name="wo", bufs=4))
                mkp = pc.enter_context(tc.tile_pool(name="mk", bufs=6))
                outp = pc.enter_context(tc.tile_pool(name="outp", bufs=1))
                psC = pc.enter_context(tc.tile_pool(name="psC", bufs=1,
                                                    space="PSUM"))
                cps = psC.tile([128, 4, D], F32)
                i = 0
                for f in range(FAN):
                    for s0 in range(0, NS, 2):
                        nt = min(2, NS - s0)
                        t0 = f * NS + s0
                        wo = wop.tile([128, nt, D], BF16, tag="wo",
                                      name=f"wo_{t0}")
                        nc.sync.dma_start(
                            wo[:],
                            WoTd[t0 * 128:(t0 + nt) * 128, :]
                            .rearrange("(u p) d -> p u d", p=128))
                        for u in range(nt):
                            s = s0 + u
                            mk = mkp.tile([128, T], BF16, tag="mk",
                                          name=f"mk_{t0 + u}")
                            nc.vector.tensor_tensor(
                                mk[:], acr[:, f * NS + s, :],
                                mask_g[:, s, :], ALU.mult)
                            for c in range(4):
                                for hh in range(2):
                                    nc.tensor.matmul(
                                        cps[:, c, hh * 512:(hh + 1) * 512],
                                        mk[:, c * 128:(c + 1) * 128],
                                        wo[:, u, hh * 512:(hh + 1) * 512],
                                        start=(i == 0), stop=(i == NRT - 1))
                            i += 1

                emit_dots(2, 0)
                emit_dots(2, 1)
                emit_accums(2, 0)
                emit_accums(2, 1)
                emit_dots(3, 0)
                emit_dots(3, 1)
                emit_accums(3, 0)
                emit_accums(3, 1)

                osb = outp.tile([128, 4, D], F32)
                for c in range(4):
                    nc.vector.tensor_tensor(osb[:, c, :], cps[:, c, :],
                                            out5[:, c, :], ALU.add)
                nc.sync.dma_start(outd.rearrange("(c p) d -> p c d", p=128),
                                  osb[:])

    nc.compile()
    return nc


_NC_CACHE = {}


def _get_nc(route_mode=ROUTE_MODE, debug=DEBUG):
    key = (route_mode, debug)
    if key not in _NC_CACHE:
        _NC_CACHE[key] = build_nc(route_mode, debug)
    return _NC_CACHE[key]


def _split_hi_lo(a):
    hi = a.astype(NPBF16)
    lo = (a - hi.astype(np.float32)).astype(NPBF16)
    return hi, lo


def _prep_inputs(oldx, W_in, b_in, W_out, route_mode):
    x = np.ascontiguousarray(np.asarray(oldx, np.float32).reshape(-1, D))
    Wr4 = np.asarray(W_in, np.float32).reshape(P, G, FAN, D)
    br = np.asarray(b_in, np.float32).reshape(P, G, FAN)
    Wo4 = np.asarray(W_out, np.float32).reshape(D, P, G, FAN)

    A = np.zeros((FAN, PLANE, D), np.float32)
    A[:, :RROWS] = Wr4[:, :RG].transpose(2, 0, 1, 3).reshape(FAN, RROWS, D)
    WrT = np.ascontiguousarray(A.reshape(FAN * PLANE, D).T)  # [D, 5632]

    b_dev = np.zeros((FAN, PLANE), np.float32)
    b_dev[:, :RROWS] = br[:, :RG].transpose(2, 0, 1).reshape(FAN, RROWS)
    bvec = np.ascontiguousarray(b_dev.reshape(NRT, 128).T)   # [128, 44]

    Wo_dev = np.zeros((FAN, PLANE, D), np.float32)
    Wo_dev[:, :RROWS] = (Wo4[:, :, :RG, :].transpose(3, 1, 2, 0)
                         .reshape(FAN, RROWS, D))
    WoT = np.ascontiguousarray(Wo_dev.reshape(FAN * PLANE, D)).astype(NPBF16)

    W5in = np.zeros((P * L5, E5), np.float32)
    W5in[:, :FAN * D] = Wr4[:, RG:].reshape(P * L5, FAN * D)
    W5in[:, FAN * D:FAN * D + FAN] = br[:, RG:].reshape(P * L5, FAN)
    W5in = W5in.astype(NPBF16)
    W5o = np.zeros((P * L5, E5), np.float32)
    W5o[:, :FAN * D] = (Wo4[:, :, RG:, :].transpose(1, 2, 3, 0)
                        .reshape(P * L5, FAN * D))
    W5o = W5o.astype(NPBF16)

    iota = np.tile(np.arange(RG, dtype=np.float32), (128, 1))
    ident = np.eye(128, dtype=np.float32).astype(NPBF16)
    xT = np.ascontiguousarray(x.T)                            # [D, B]

    shared = {"bvec": bvec, "WoT": WoT, "W5in": W5in, "W5o": W5o,
              "ident": ident, "iota": iota}
    in_maps = []
    for cc in range(NCORES):
        m = dict(shared)
        xs = np.ascontiguousarray(xT[:, cc * T:(cc + 1) * T])
        m["xtok"] = np.ascontiguousarray(x[cc * T:(cc + 1) * T, :]
                                         ).astype(NPBF16)
        if route_mode == "f32r":
            m["xT"] = xs
            m["Wr"] = WrT
        else:
            m["xhi"], m["xlo"] = _split_hi_lo(xs)
            m["Whi"], m["Wlo"] = _split_hi_lo(WrT)
        in_maps.append(m)
    return in_maps


_WARM = False


def run(oldx, W_in, b_in, W_out, trace=False, route_mode=ROUTE_MODE,
        debug=DEBUG):
    nc = _get_nc(route_mode, debug)
    in_maps = _prep_inputs(oldx, W_in, b_in, W_out, route_mode)
    global _WARM
    if not _WARM:
        run_bass_kernel_spmd(nc, in_maps, list(range(NCORES)), trace=False)
        _WARM = True
    res = run_bass_kernel_spmd(nc, in_maps, list(range(NCORES)), trace=trace)
    out = np.concatenate([res.results[c]["outT"] for c in range(NCORES)],
                         axis=0)
    return np.ascontiguousarray(out).reshape(np.asarray(oldx).shape), res


def kernel(oldx, W_in, b_in, W_out):
    out, _ = run(oldx, W_in, b_in, W_out, trace=False)
    return out
